# revision 30
# baseline (speedup 1.0000x reference)
"""Trainium2 Bass kernel for the 3-layer GAT + path-CNN model (nn_GAT_41729902248227).

Node-sharded graph parallelism over 8 NeuronCores:
 - Nodes sharded contiguously (N/8 per core, padded to 49 blocks of 128).
 - Edges sharded by dst, sorted by dst, grouped per 128-dst-node block.
   Within a block, edges are split by src table half (int16 gather index
   limit) and padded to a uniform per-half chunk count (KS) so a single SPMD
   program serves all cores.
 - Per GAT layer: a dense phase computes [f | el | er] = h @ [W | W@Al | W@Ar]
   per 128-node tile (one matmul) and writes 768B rows to a DRAM table that is
   AllGathered. The edge phase uses the dma_gather Q7 ucode: per block, two
   row gathers fetch f/el by src (lo/hi table halves) and one narrow-column
   gather fetches er by dst from the LOCAL table (dst indices are core-local
   so they fit int16 without splitting). One-hot dst-selection matrices are
   built on-device (iota + is_equal) and the edge softmax + aggregation folds
   into PE matmuls accumulating [sum ex*f | sum ex] in PSUM, followed by
   per-head normalization and ELU.
 - CNN head: logits stored bf16 pair-packed (two nodes per 256B row) so the
   row count fits int16; one dma_gather per 64-node supertile fetches all path
   rows, a predicated copy selects the node half, and conv1/conv2 run as bf16
   matmuls with host-expanded block weights using PE row/col tile grouping.
"""

import sys

sys.path.insert(0, "/opt/trn_rl_repo")

from contextlib import ExitStack

import ml_dtypes
import numpy as np

import concourse.bacc as bacc
import concourse.bass as bass
import concourse.mybir as mybir
import concourse.tile as tile
from concourse import bass_utils
from concourse.library_config import mlp

F32 = mybir.dt.float32
BF16 = mybir.dt.bfloat16
FP16 = mybir.dt.float16
I16 = mybir.dt.int16
AF = mybir.ActivationFunctionType
OP = mybir.AluOpType
NPBF16 = ml_dtypes.bfloat16

P = 128
GAT_NEG = 0.2
CNN_NEG = 0.01
# fe table rows are fp16 slots; el/er stay f32, bit-packed into pairs of slots.
ROW = 256    # layers 0/1: [f 0:128 | el(f32) 128:136 | er(f32) 136:144 | pad] = 512B
ROW2 = 128   # layer 2:    [f2 0:64 | el2(f32) 64:66 | er2(f32) 66:68 | pad] = 256B


class Cfg:
    def __init__(self, n_nodes, n_cores, in_dim, n_hid, h_hid, n_cls, cnn_ch,
                 groups, kc):
        self.N = n_nodes
        self.C = n_cores
        self.NLOC = n_nodes // n_cores
        self.NBLK = (self.NLOC + P - 1) // P
        self.NPAD = self.NBLK * P
        self.IN_DIM = in_dim          # 512
        self.H = h_hid                # 4 heads
        self.D = n_hid                # 32 per head
        self.F = h_hid * n_hid        # 128
        self.NCLS = n_cls             # 64
        self.CNN_CH = cnn_ch          # 32
        # f-gather call plan: list of (start_chunk, n_chunks, table_base_row).
        # Each call's indices are int16 offsets from table_base_row, so a call
        # window covers 32768 rows; windows overlap, edges are assigned to
        # whichever call has room.
        self.GROUPS = groups
        self.K = sum(g[1] for g in groups)   # chunks per block
        self.KC = kc                  # CNN chunks per 64-node supertile (=32)
        self.KIN = in_dim // P
        self.ST = self.NPAD // 64
        self.NG = n_cores * self.NPAD
        self.NGH = self.NG // 2


def build_program(cfg: Cfg, use_act_lrelu: bool = True, max_stage: int = 99):
    c = cfg
    nc = bacc.Bacc("TRN2", target_bir_lowering=False, debug=False,
                   enable_asserts=False, num_devices=c.C, num_swdge_queues=4)

    def nextq():
        return 0

    CH3 = 3 * c.CNN_CH  # 96
    K, KC = c.K, c.KC

    # ---- external inputs ----
    xT_d = nc.dram_tensor("xT", [c.KIN, P, c.NPAD], F32, kind="ExternalInput")
    wcat0_d = nc.dram_tensor("wcat0", [c.KIN, P, c.F + 8], F32, kind="ExternalInput")
    wcat1_d = nc.dram_tensor("wcat1", [P, c.F + 8], F32, kind="ExternalInput")
    wcat2_d = nc.dram_tensor("wcat2", [c.F, c.NCLS + 2], F32, kind="ExternalInput")
    eidx_d = nc.dram_tensor("eidx", [c.NBLK, P, K * 8], I16, kind="ExternalInput")
    edstv_d = nc.dram_tensor("edstv", [c.NBLK, P, K], F32, kind="ExternalInput")
    # transposed one-hot dst-selection matrix: sth[v, k*P+p] = 1 iff edge
    # slot (p,k) of the block has dst == v (used for the er broadcast)
    sth_d = nc.dram_tensor("sth", [c.NBLK, P, K * P], FP16,
                           kind="ExternalInput")
    cidx_d = nc.dram_tensor("cidx", [c.ST, P, KC * 8], I16, kind="ExternalInput")
    cmsk_d = nc.dram_tensor("cmsk", [c.ST, P, 2 * KC], FP16, kind="ExternalInput")
    cw1ab_d = nc.dram_tensor("cw1ab", [P, 2 * CH3], FP16, kind="ExternalInput")
    cb1r_d = nc.dram_tensor("cb1r", [CH3, 1], F32, kind="ExternalInput")
    w2c_d = nc.dram_tensor("w2c", [CH3, 32], FP16, kind="ExternalInput")
    cb2_d = nc.dram_tensor("cb2", [P, 1], F32, kind="ExternalInput")

    y_d = nc.dram_tensor("y", [c.NPAD, c.NCLS], F32, kind="ExternalOutput")

    # ---- internal DRAM ----
    fe_loc = [nc.dram_tensor("fe_loc0", [c.NPAD, ROW], FP16),
              nc.dram_tensor("fe_loc1", [c.NPAD, ROW], FP16),
              nc.dram_tensor("fe_loc2", [c.NPAD, ROW2], FP16)]
    fe_full = [nc.dram_tensor("fe_full0", [c.NG, ROW], FP16, addr_space="Shared"),
               nc.dram_tensor("fe_full1", [c.NG, ROW], FP16, addr_space="Shared"),
               nc.dram_tensor("fe_full2", [c.NG, ROW2], FP16, addr_space="Shared")]
    lg_loc = nc.dram_tensor("lg_loc", [c.NPAD // 2, P], FP16)
    lg_full = nc.dram_tensor("lg_full", [c.NG // 2, P], FP16, addr_space="Shared")

    groups = [list(range(c.C))]
    h_tiles = {}

    with tile.TileContext(nc) as tc:
        with ExitStack() as ctx:
            consts = ctx.enter_context(tc.tile_pool(name="consts", bufs=1))
            hpool = ctx.enter_context(tc.tile_pool(name="hpool", bufs=1))

            nc.gpsimd.load_library(mlp)

            iota_t = consts.tile([P, P], F32)
            nc.gpsimd.iota(iota_t[:], pattern=[[1, P]], base=0, channel_multiplier=0,
                           allow_small_or_imprecise_dtypes=True)
            from concourse.masks import make_identity
            ident_t = consts.tile([P, P], F32)
            make_identity(nc, ident_t[:])
            wcat0_t = consts.tile([P, c.KIN * (c.F + 8)], F32)
            nc.sync.dma_start(out=wcat0_t[:].rearrange("p (k w) -> p k w", k=c.KIN),
                              in_=wcat0_d[:, :, :].transpose([1, 0, 2]))
            wcat1_t = consts.tile([P, c.F + 8], F32)
            nc.sync.dma_start(out=wcat1_t[:], in_=wcat1_d[:, :])
            wcat2_t = consts.tile([c.F, c.NCLS + 2], F32)
            nc.sync.dma_start(out=wcat2_t[:], in_=wcat2_d[:, :])
            cw1ab_t = consts.tile([P, 2 * CH3], FP16)
            nc.sync.dma_start(out=cw1ab_t[:], in_=cw1ab_d[:, :])
            cb1r_t = consts.tile([CH3, 1], F32)
            nc.sync.dma_start(out=cb1r_t[:], in_=cb1r_d[:, :])
            w2c_t = consts.tile([CH3, 32], FP16)
            nc.sync.dma_start(out=w2c_t[:], in_=w2c_d[:, :])
            cb2_t = consts.tile([P, 1], F32)
            nc.sync.dma_start(out=cb2_t[:], in_=cb2_d[:, :])

            def all_gather(src_t, dst_t):
                nc.gpsimd.collective_compute(
                    "AllGather", OP.bypass, replica_groups=groups,
                    ins=[src_t.ap().opt()], outs=[dst_t.ap().opt()])

            with ExitStack() as gctx:
                sbA = gctx.enter_context(tc.tile_pool(name="sbA", bufs=2))
                psA = gctx.enter_context(tc.tile_pool(name="psA", bufs=2, space="PSUM"))
                sbC = gctx.enter_context(tc.tile_pool(name="sbC", bufs=2))
                psC = gctx.enter_context(tc.tile_pool(name="psC", bufs=2, space="PSUM"))

                def phase_a(layer):
                    fdim = c.F if layer < 2 else c.NCLS
                    wcols = fdim + 8 if layer < 2 else fdim + 2
                    for t in range(c.NBLK):
                        pA = psA.tile([P, c.F + 8], F32, tag="pA", space="PSUM")
                        if layer == 0:
                            xk = sbA.tile([P, c.KIN * P], F32, tag="xk")
                            nc.sync.dma_start(
                                out=xk[:].rearrange("p (k n) -> p k n", k=c.KIN),
                                in_=xT_d[:, :, t * P:(t + 1) * P].transpose([1, 0, 2]))
                            for k in range(c.KIN):
                                nc.tensor.matmul(
                                    out=pA[:, 0:wcols],
                                    lhsT=xk[:, k * P:(k + 1) * P],
                                    rhs=wcat0_t[:, k * wcols:(k + 1) * wcols],
                                    start=(k == 0), stop=(k == c.KIN - 1))
                        else:
                            hin = h_tiles[(layer, t)]
                            pT = psA.tile([P, P], F32, tag="pT", space="PSUM")
                            nc.tensor.transpose(out=pT[:], in_=hin[:],
                                                identity=ident_t[:])
                            hT = sbA.tile([P, P], F32, tag="hT")
                            nc.vector.tensor_copy(out=hT[:], in_=pT[:])
                            nc.tensor.matmul(
                                out=pA[:, 0:wcols], lhsT=hT[:],
                                rhs=(wcat1_t[:] if layer == 1 else wcat2_t[:]),
                                start=True, stop=True)
                        # keep er for this block resident in SBUF (fp16): the
                        # edge phase broadcasts it to edge slots via matmul
                        nh2 = c.H if layer < 2 else 1
                        ersb = hpool.tile([P, 8], FP16, tag=f"er{layer}_{t}")
                        nc.vector.tensor_copy(
                            out=ersb[:, 0:nh2],
                            in_=pA[:, fdim + nh2:fdim + 2 * nh2])
                        h_tiles[(f"er{layer}", t)] = ersb
                        fdim_ = fdim
                        nsl = fdim_ + 2 * (wcols - fdim_)  # fp16 slots used
                        fea = sbA.tile([P, c.F + 16], FP16, tag="fea")
                        nc.vector.tensor_copy(out=fea[:, 0:fdim_], in_=pA[:, 0:fdim_])
                        nc.vector.tensor_copy(
                            out=fea[:, fdim_:nsl].bitcast(F32),
                            in_=pA[:, fdim_:wcols])
                        nc.sync.dma_start(out=fe_loc[layer][t * P:(t + 1) * P, 0:nsl],
                                          in_=fea[:, 0:nsl])

                def phase_c(layer):
                    fdim = c.F if layer < 2 else c.NCLS      # 128 / 64
                    nh = c.H if layer < 2 else 1
                    rb = ROW if layer < 2 else ROW2
                    ecol = fdim  # el col offset within row
                    for b in range(c.NBLK):
                        idx = sbC.tile([P, K * 8], I16, tag="idx")
                        nc.sync.dma_start(out=idx[:], in_=eidx_d[b, :, :])
                        dstv = sbC.tile([P, K], F32, tag="dstv")
                        nc.sync.dma_start(out=dstv[:], in_=edstv_d[b, :, :])
                        sth = sbC.tile([P, K * P], FP16, tag="sth")
                        nc.sync.dma_start(out=sth[:], in_=sth_d[b, :, :])
                        feg = sbC.tile([P, K * ROW], FP16, tag="feg")
                        feg3 = feg[:, 0:K * rb].rearrange("p (k r) -> p k r", r=rb)
                        for gs, gnk, gbase in c.GROUPS:
                            nc.gpsimd.dma_gather(
                                feg3[:, gs:gs + gnk, :],
                                fe_full[layer][gbase:c.NG, :],
                                idx[:, gs * 8:(gs + gnk) * 8],
                                gnk * P, gnk * P, rb, queue_num=nextq())
                        # er_edge[p, (k,h)] = er_blk[dstv[p,k], h] via K matmuls
                        ersb = h_tiles[(f"er{layer}", b)]
                        erp = psC.tile([P, K * c.H], F32, tag="erp", space="PSUM")
                        for k in range(K):
                            nc.tensor.matmul(
                                out=erp[:, k * nh:k * nh + nh],
                                lhsT=sth[:, k * P:(k + 1) * P],
                                rhs=ersb[:, 0:nh], start=True, stop=True)
                        # f32 view of el (in feg)
                        el_f32 = feg3[:, :, fdim:fdim + 2 * nh].bitcast(F32)
                        # S[p, k, v] = (v == dstv[p, k])
                        s_all = sbC.tile([P, K * P], FP16, tag="sall")
                        nc.vector.tensor_tensor(
                            out=s_all[:].rearrange("p (k v) -> p k v", v=P),
                            in0=iota_t[:].unsqueeze(1).to_broadcast([P, K, P]),
                            in1=dstv[:].unsqueeze(2).to_broadcast([P, K, P]),
                            op=OP.is_equal)
                        # e = lrelu(el + er); ex = exp(e) overwrites el cols in feg
                        e_t = sbC.tile([P, K * c.H], F32, tag="e_t")
                        e3 = e_t[:, 0:K * nh].rearrange("p (k h) -> p k h", h=nh)
                        nc.vector.tensor_tensor(
                            out=e3, in0=el_f32,
                            in1=erp[:, 0:K * nh].rearrange("p (k h) -> p k h", h=nh),
                            op=OP.add)
                        nc.vector.scalar_tensor_tensor(
                            out=e_t[:, 0:K * nh], in0=e_t[:, 0:K * nh], scalar=GAT_NEG,
                            in1=e_t[:, 0:K * nh], op0=OP.mult, op1=OP.max)
                        nc.scalar.activation(out=feg3[:, :, ecol:ecol + nh], in_=e3,
                                             func=AF.Exp)
                        # exe = exp(e) pre-expanded along D on the (idle) scalar
                        # engine so the g-multiply is a contiguous fp16 DVE op
                        dph = c.D if layer < 2 else fdim
                        exe = sbC.tile([P, K * c.F], FP16, tag="exe")
                        exe4 = exe[:, 0:K * fdim].rearrange(
                            "p (k h d) -> p k h d", k=K, d=dph)
                        nc.scalar.activation(
                            out=exe4,
                            in_=e3.unsqueeze(3).to_broadcast([P, K, nh, dph]),
                            func=AF.Exp)
                        # g = f * ex, in place
                        nc.vector.tensor_tensor(
                            out=feg3[:, :, 0:fdim], in0=feg3[:, :, 0:fdim],
                            in1=exe[:, 0:K * fdim].rearrange(
                                "p (k d) -> p k d", k=K),
                            op=OP.mult)
                        gw = fdim + nh
                        agg = psC.tile([P, c.F + c.H], F32, tag="agg", space="PSUM")
                        for k in range(K):
                            nc.tensor.matmul(
                                out=agg[:, 0:gw],
                                lhsT=s_all[:, k * P:(k + 1) * P],
                                rhs=feg[:, k * rb:k * rb + gw],
                                start=(k == 0), stop=(k == K - 1))
                        rec = sbC.tile([P, c.H], F32, tag="rec")
                        nc.vector.tensor_scalar_max(rec[:, 0:nh],
                                                    agg[:, fdim:fdim + nh], 1e-30)
                        nc.vector.reciprocal(out=rec[:, 0:nh], in_=rec[:, 0:nh])
                        if layer < 2:
                            z = sbC.tile([P, c.F], F32, tag="z")
                            for h in range(nh):
                                nc.vector.tensor_scalar_mul(
                                    z[:, h * c.D:(h + 1) * c.D],
                                    agg[:, h * c.D:(h + 1) * c.D], rec[:, h:h + 1])
                            zm = sbC.tile([P, c.F], F32, tag="zm")
                            nc.vector.tensor_scalar_min(zm[:], z[:], 0.0)
                            ze = sbC.tile([P, c.F], F32, tag="ze")
                            nc.scalar.activation(out=ze[:], in_=zm[:], func=AF.Exp)
                            nc.vector.tensor_scalar_max(z[:], z[:], 0.0)
                            hout = hpool.tile([P, c.F], F32, tag=f"h{layer + 1}_{b}")
                            nc.vector.scalar_tensor_tensor(
                                out=hout[:], in0=ze[:], scalar=-1.0, in1=z[:],
                                op0=OP.add, op1=OP.add)
                            h_tiles[(layer + 1, b)] = hout
                        else:
                            lgt = sbC.tile([P, c.NCLS], F32, tag="lgt")
                            nc.vector.tensor_scalar_mul(lgt[:], agg[:, 0:c.NCLS],
                                                        rec[:, 0:1])
                            lgb = sbC.tile([P, c.NCLS], FP16, tag="lgb")
                            nc.vector.tensor_copy(out=lgb[:], in_=lgt[:])
                            nc.sync.dma_start(
                                out=lg_loc[b * 64:(b + 1) * 64, :]
                                    .rearrange("r (a d) -> (r a) d", a=2),
                                in_=lgb[:])

                for layer in range(3):
                    if max_stage < 2 * layer + 1:
                        break
                    phase_a(layer)
                    all_gather(fe_loc[layer], fe_full[layer])
                    if max_stage < 2 * layer + 2:
                        break
                    phase_c(layer)

                if max_stage >= 7:
                    all_gather(lg_loc, lg_full)

            # ---------------- CNN head ----------------
            with ExitStack() as dctx:
                sbD = dctx.enter_context(tc.tile_pool(name="sbD", bufs=2))
                sbD3 = dctx.enter_context(tc.tile_pool(name="sbD3", bufs=3))
                psD = dctx.enter_context(tc.tile_pool(name="psD", bufs=2, space="PSUM"))

                for st in range(c.ST if max_stage >= 8 else 0):
                    cidx = sbD.tile([P, KC * 8], I16, tag="cidx")
                    nc.sync.dma_start(out=cidx[:], in_=cidx_d[st, :, :])
                    cmsk = sbD.tile([P, 2 * KC], FP16, tag="cmsk")
                    nc.sync.dma_start(out=cmsk[:], in_=cmsk_d[st, :, :])
                    for g in range(KC // 8 if max_stage >= 9 else 0):
                        o = g * 8
                        # per-group gather + select so group g's compute
                        # overlaps group g+1's gather DMA
                        Xg = sbD3.tile([P, 8 * P], FP16, tag=f"X{g}")
                        X3 = Xg[:].rearrange("p (k r) -> p k r", r=P)
                        nc.gpsimd.dma_gather(
                            X3[:, 0:8, :], lg_full[:, :],
                            cidx[:, o * 8:(o + 8) * 8], 8 * P, 8 * P, P,
                            queue_num=nextq())
                        X64 = sbD3.tile([P, 8 * 64], FP16, tag=f"X64{g}")
                        X64_3 = X64[:].rearrange("p (k r) -> p k r", r=64)
                        Xo = sbD3.tile([P, 8 * 64], FP16, tag=f"Xo{g}")
                        Xo_3 = Xo[:].rearrange("p (k r) -> p k r", r=64)
                        # exact parity select: X64 = Xl*(1-m) + Xr*m (masks 0/1)
                        nc.vector.tensor_tensor(
                            out=X64_3, in0=X3[:, :, 0:64],
                            in1=cmsk[:, o:o + 8].unsqueeze(2)
                                .to_broadcast([P, 8, 64]),
                            op=OP.mult)
                        nc.vector.tensor_tensor(
                            out=Xo_3, in0=X3[:, :, 64:P],
                            in1=cmsk[:, KC + o:KC + o + 8].unsqueeze(2)
                                .to_broadcast([P, 8, 64]),
                            op=OP.mult)
                        nc.vector.tensor_tensor(out=X64[:], in0=X64[:],
                                                in1=Xo[:], op=OP.add)
                        y2p = psD.tile([64, 512], F32, tag="y2p", space="PSUM")
                        for par in range(2):
                            y1p = psD.tile([CH3, 512], F32, tag=f"y1p{par}",
                                           space="PSUM")
                            nc.tensor.matmul(
                                out=y1p[:],
                                lhsT=cw1ab_t[par * 64:(par + 1) * 64,
                                             par * CH3:(par + 1) * CH3],
                                rhs=X64[par * 64:(par + 1) * 64, :],
                                start=True, stop=True)
                            y1s = sbD3.tile([CH3, 512], FP16, tag=f"y1s{par}")
                            if use_act_lrelu:
                                nc.scalar.activation(out=y1s[:], in_=y1p[:],
                                                     func=AF.Lrelu,
                                                     bias=cb1r_t[:, 0:1],
                                                     alpha=CNN_NEG)
                            else:
                                yt = sbD3.tile([CH3, 512], F32, tag=f"yt{par}")
                                nc.vector.tensor_scalar_add(yt[:], y1p[:],
                                                            cb1r_t[:, 0:1])
                                nc.vector.scalar_tensor_tensor(
                                    out=y1s[:], in0=yt[:], scalar=CNN_NEG, in1=yt[:],
                                    op0=OP.mult, op1=OP.max)
                            nc.tensor.matmul(out=y2p[par * 32:(par + 1) * 32, :],
                                             lhsT=w2c_t[:], rhs=y1s[:],
                                             start=True, stop=True,
                                             tile_position=(0, par * 32))
                        ysb = sbD.tile([64, 512], F32, tag="ysb")
                        if use_act_lrelu:
                            nc.scalar.activation(out=ysb[:], in_=y2p[:], func=AF.Lrelu,
                                                 bias=cb2_t[0:64, 0:1], alpha=CNN_NEG)
                        else:
                            nc.vector.tensor_scalar_add(ysb[:], y2p[:],
                                                        cb2_t[0:64, 0:1])
                            nc.vector.scalar_tensor_tensor(
                                out=ysb[:], in0=ysb[:], scalar=CNN_NEG, in1=ysb[:],
                                op0=OP.mult, op1=OP.max)
                        r0 = st * 64 + g * 16
                        nc.sync.dma_start(
                            out=y_d[r0:r0 + 16, :].rearrange("(b a) d -> a b d", a=2),
                            in_=ysb[0:64:32, :].rearrange("a (b d) -> a b d", d=c.NCLS))
    # Align SWDGE queue assignment with Tile's DMASW sem-lane rotation:
    # sem lane s must always service the same queue, so queue = lane % 4.
    from concourse.tile_sem_assignment import PROC_NAME_TO_IDX
    lane_to_q = {}
    for q in range(8):
        if f"DMASW{q}" in PROC_NAME_TO_IDX:
            lane_to_q[PROC_NAME_TO_IDX[f"DMASW{q}"]] = q % 4
    nfix = 0
    for blk in nc.m.functions[0].blocks:
        for inst in blk.instructions:
            if isinstance(inst, mybir.InstDMAGatherAnt):
                proc = inst.bass_scheduled_proc
                if proc in lane_to_q:
                    inst.queue_num = lane_to_q[proc]
                    nfix += 1
    nc.compile()
    return nc


# ------------------------------------------------------------------
# host-side preparation
# ------------------------------------------------------------------

def _block_diag_a(a):
    a = np.asarray(a, np.float32)
    hh, dd = a.shape
    out = np.zeros((hh * dd, hh), np.float32)
    for h in range(hh):
        out[h * dd:(h + 1) * dd, h] = a[h]
    return out


def _wrap16(slots, ncols):
    """slots: int array [NI] -> [128, NI//16] int16, wrapped in 16 partitions
    and replicated into all 8 16-partition groups."""
    ni = slots.shape[0]
    m = np.zeros((P, ncols), np.int16)
    j = np.arange(ni)
    for rep in range(8):
        m[j % 16 + 16 * rep, j // 16] = slots.astype(np.int16)
    return m


def compute_groups(nloc, dst, ncores):
    """f-gather call plan from the worst per-block edge count.

    Calls are capped at 1024 indices (8 chunks) by the gather ucode ring.
    Call 0 addresses table rows [0, 32768), call 1 rows [NG-32768, NG);
    overflow calls address [0, 32768) again (they only ever hold edges
    whose src row < 32768).
    """
    nblk = (nloc + P - 1) // P
    worst = 0
    for core in range(ncores):
        lo = core * nloc
        m = (dst >= lo) & (dst < lo + nloc)
        cnt = np.bincount((dst[m] - lo) // P, minlength=nblk)
        worst = max(worst, int(cnt.max()))
    npad = nblk * P
    base2 = ncores * npad - 32768
    if worst <= 1024:
        return [(0, (worst + P - 1) // P, 0)], base2
    groups = [(0, 8, 0), (8, 8, base2)]
    left = worst - 2048
    gs = 16
    while left > 0:
        gnk = min(8, (left + P - 1) // P)
        groups.append((gs, gnk, 0))
        gs += gnk
        left -= gnk * P
    return groups, base2


def prepare_inputs(cfg, x, src, dst, paths, W0, al0, ar0, W1, al1, ar1,
                   W2, al2, ar2, cw1, cb1, cw2, cb2):
    c = cfg
    x = np.asarray(x, np.float32)
    src = np.asarray(src, np.int64)
    dst = np.asarray(dst, np.int64)
    paths = np.asarray(paths, np.int64)
    W0 = np.asarray(W0, np.float32)
    W1 = np.asarray(W1, np.float32)
    W2 = np.asarray(W2, np.float32)
    al2 = np.asarray(al2, np.float32)
    ar2 = np.asarray(ar2, np.float32)
    cw1 = np.asarray(cw1, np.float32)
    cw2 = np.asarray(cw2, np.float32)

    wcat0 = np.concatenate(
        [W0, W0 @ _block_diag_a(al0), W0 @ _block_diag_a(ar0)], axis=1
    ).astype(np.float32).reshape(c.KIN, P, c.F + 8)
    wcat1 = np.ascontiguousarray(np.concatenate(
        [W1, W1 @ _block_diag_a(al1), W1 @ _block_diag_a(ar1)], axis=1
    ).astype(np.float32))
    wcat2 = np.ascontiguousarray(np.concatenate(
        [W2, W2 @ al2.reshape(-1, 1), W2 @ ar2.reshape(-1, 1)], axis=1
    ).astype(np.float32))

    ch = c.CNN_CH
    CH3 = 3 * ch
    cw1e = np.zeros((50, CH3), np.float32)
    for co in range(ch):
        for pp in range(3):
            for ci in range(10):
                for t in range(3):
                    cw1e[ci * 5 + pp + t, co * 3 + pp] = cw1[co, ci, t, 0]
    cw1ab = np.zeros((P, 2 * CH3), np.float32)
    cw1ab[0:50, 0:CH3] = cw1e
    cw1ab[64:114, CH3:2 * CH3] = cw1e
    cb1r = np.repeat(np.asarray(cb1, np.float32), 3).reshape(CH3, 1)
    w2c = np.zeros((CH3, 32), np.float32)
    for co in range(ch):
        for pp in range(3):
            w2c[co * 3 + pp, :] = cw2[0, co, pp, 0]
    cb2a = np.full((P, 1), np.float32(np.asarray(cb2).reshape(-1)[0]))

    def grow(v):
        return (v // c.NLOC) * c.NPAD + (v % c.NLOC)

    in_maps = []
    for core in range(c.C):
        lo, hi = core * c.NLOC, (core + 1) * c.NLOC
        xs = np.zeros((c.NPAD, c.IN_DIM), np.float32)
        xs[:c.NLOC] = x[lo:hi]
        xT = np.ascontiguousarray(xs.T).reshape(c.KIN, P, c.NPAD)

        m = (dst >= lo) & (dst < hi)
        es, ed = grow(src[m]), dst[m] - lo
        order = np.argsort(ed, kind="stable")
        es, ed = es[order], ed[order]
        eblk = ed // P
        eidx = np.zeros((c.NBLK, P, c.K * 8), np.int16)
        edstv = np.full((c.NBLK, P, c.K), -1.0, np.float32)
        sthf = np.zeros((c.NBLK, P, c.K * P), np.float16)
        base2 = c.NG - 32768  # second-window base; window = [base2, NG)
        for b in range(c.NBLK):
            bm = eblk == b
            bs, bd = es[bm], ed[bm]
            # assign edges to gather calls: call 0 covers rows [0, 32768),
            # call 1 covers [base2, NG), extra calls cover [0, 32768).
            caps = [gnk * P for _, gnk, _ in c.GROUPS]
            if len(c.GROUPS) == 1:
                assert len(bs) <= caps[0]
                asn = [np.arange(len(bs))]
            else:
                i1 = np.where(bs >= 32768)[0]       # must use window-1 call
                i0 = np.where(bs < base2)[0]        # must use a base-0 call
                iflex = np.where((bs >= base2) & (bs < 32768))[0]
                assert len(i1) <= caps[1], (core, b, len(i1))
                nf2 = min(caps[1] - len(i1), len(iflex))
                asn = [None] * len(c.GROUPS)
                asn[1] = np.concatenate([i1, iflex[:nf2]])
                rest = np.concatenate([i0, iflex[nf2:]]).astype(np.int64)
                asn[0] = rest[:caps[0]]
                rest = rest[caps[0]:]
                for gi in range(2, len(c.GROUPS)):
                    asn[gi] = rest[:caps[gi]]
                    rest = rest[caps[gi]:]
                assert rest.size == 0, (core, b, rest.size)
            for gi, (gstart, gnk, gbase) in enumerate(c.GROUPS):
                sel = asn[gi]
                hs, hd = bs[sel], bd[sel]
                ne = hs.shape[0]
                # pad slots point at spread-out (valid) rows: same-address
                # row-0 fetches serialize on one HBM bank and skew the DMA
                # engine load badly
                g_idx = (np.arange(gnk * P, dtype=np.int64) * 997) % 16384
                g_idx[:ne] = hs - gbase
                assert ne == 0 or (g_idx[:ne].min() >= 0
                                   and g_idx[:ne].max() < 32768)
                j = np.arange(ne)
                # slot j of this call -> chunk gstart + j//128, partition j%128
                hdr = (hd - b * P).astype(np.int64)
                edstv[b, j % P, gstart + j // P] = hdr.astype(np.float32)
                slot = (gstart + j // P) * P + j % P
                sthf[b, hdr, slot] = np.float16(1.0)
                eidx[b, :, gstart * 8:(gstart + gnk) * 8] = _wrap16(
                    g_idx, gnk * 8)


        pl = np.zeros((c.NPAD, 50), np.int64)
        pl[:c.NLOC] = paths[lo:hi].reshape(c.NLOC, 50)
        pg = grow(pl)                                        # [NPAD, 50]
        cidx = np.zeros((c.ST, P, c.KC * 8), np.int16)
        cmsk = np.zeros((c.ST, P, 2 * c.KC), np.float16)
        for st in range(c.ST):
            slots = np.zeros((c.KC * P,), np.int64)
            j = np.arange(c.KC * P)
            k, p = j // P, j % P
            n = st * 64 + 2 * k + (p >= 64)
            q = p % 64
            valid = q < 50
            v = np.where(valid, pg[n, np.minimum(q, 49)],
                         (j * 1997) % (c.NG // 2) * 2)
            slots[j] = v >> 1
            cidx[st] = _wrap16(slots, c.KC * 8)
            cmsk[st, p, k] = (1 - (v & 1)).astype(np.float16)
            cmsk[st, p, c.KC + k] = (v & 1).astype(np.float16)
        in_maps.append({
            "xT": xT, "wcat0": wcat0, "wcat1": wcat1, "wcat2": wcat2,
            "eidx": eidx, "edstv": edstv, "sth": sthf,
            "cidx": cidx, "cmsk": cmsk,
            "cw1ab": cw1ab.astype(np.float16), "cb1r": cb1r,
            "w2c": w2c.astype(np.float16), "cb2": cb2a,
        })
    return in_maps


_CACHE = {}


def kernel(**inputs):
    x = np.asarray(inputs["x"], np.float32)
    n_nodes = x.shape[0]
    dst = np.asarray(inputs["dst"], np.int64)
    ncores = 8
    nloc = n_nodes // ncores
    groups, _ = compute_groups(nloc, dst, ncores)
    cfg = Cfg(n_nodes=n_nodes, n_cores=ncores, in_dim=x.shape[1],
              n_hid=32, h_hid=4, n_cls=64,
              cnn_ch=int(np.asarray(inputs["cw1"]).shape[0]),
              groups=groups, kc=32)

    key = (n_nodes, cfg.IN_DIM, tuple(groups))
    if key not in _CACHE:
        _CACHE[key] = build_program(cfg)
    nc = _CACHE[key]

    in_maps = prepare_inputs(cfg, **inputs)
    res = bass_utils.run_bass_kernel_spmd(nc, in_maps, core_ids=list(range(cfg.C)))
    y = np.concatenate(
        [res.results[core]["y"][:cfg.NLOC] for core in range(cfg.C)], axis=0)
    return np.ascontiguousarray(y.astype(np.float32))



# revision 48
# speedup vs baseline: 1.1582x; 1.1582x over previous
"""Trainium2 Bass kernel for the 3-layer GAT + path-CNN model (nn_GAT_41729902248227).

Node-sharded graph parallelism over 8 NeuronCores:
 - Nodes sharded contiguously (N/8 per core, padded to 49 blocks of 128).
 - Edges sharded by dst, sorted by dst, grouped per 128-dst-node block.
   Within a block, edges are split by src table half (int16 gather index
   limit) and padded to a uniform per-half chunk count (KS) so a single SPMD
   program serves all cores.
 - Per GAT layer: a dense phase computes [f | el | er] = h @ [W | W@Al | W@Ar]
   per 128-node tile (one matmul) and writes 768B rows to a DRAM table that is
   AllGathered. The edge phase uses the dma_gather Q7 ucode: per block, two
   row gathers fetch f/el by src (lo/hi table halves) and one narrow-column
   gather fetches er by dst from the LOCAL table (dst indices are core-local
   so they fit int16 without splitting). One-hot dst-selection matrices are
   built on-device (iota + is_equal) and the edge softmax + aggregation folds
   into PE matmuls accumulating [sum ex*f | sum ex] in PSUM, followed by
   per-head normalization and ELU.
 - CNN head: logits stored bf16 pair-packed (two nodes per 256B row) so the
   row count fits int16; one dma_gather per 64-node supertile fetches all path
   rows, a predicated copy selects the node half, and conv1/conv2 run as bf16
   matmuls with host-expanded block weights using PE row/col tile grouping.
"""

import sys

sys.path.insert(0, "/opt/trn_rl_repo")

from contextlib import ExitStack

import ml_dtypes
import numpy as np

import concourse.bacc as bacc
import concourse.bass as bass
import concourse.mybir as mybir
import concourse.tile as tile
from concourse import bass_utils
from concourse.library_config import mlp

F32 = mybir.dt.float32
BF16 = mybir.dt.bfloat16
FP16 = mybir.dt.float16
I16 = mybir.dt.int16
AF = mybir.ActivationFunctionType
OP = mybir.AluOpType
NPBF16 = ml_dtypes.bfloat16

P = 128
GAT_NEG = 0.2
CNN_NEG = 0.01
AGCH_FE = 8    # blocks per fe AllGather chunk
AGCH_LG = 16   # blocks per lg AllGather chunk


def _chunk_spans(nblk, agch):
    """[(first_block, n_blocks, cumulative_blocks_before), ...]"""
    out = []
    base = 0
    for j0 in range(0, nblk, agch):
        nb = min(agch, nblk - j0)
        out.append((j0, nb, base))
        base += nb
    return out


def _chunk_map(nblk, agch, nloc, ncores, rows_per_block):
    """Vectorized global-node-id -> chunk-major full-table row id."""
    spans = _chunk_spans(nblk, agch)
    nch = len(spans)
    j0s = np.zeros(nch, np.int64)
    nbs = np.zeros(nch, np.int64)
    cums = np.zeros(nch, np.int64)
    for i, (j0, nb, cum) in enumerate(spans):
        j0s[i], nbs[i], cums[i] = j0, nb, cum
    rb = rows_per_block

    def f(v):
        cc, r = v // nloc, v % nloc
        j = (r // P) // agch
        off = (r - j0s[j] * P) * rb // P
        return cums[j] * ncores * rb + cc * nbs[j] * rb + off

    return f
# fe table rows are fp16 slots; el/er stay f32, bit-packed into pairs of slots.
ROW = 256    # layers 0/1: [f 0:128 | el(f32) 128:136 | er(f32) 136:144 | pad] = 512B
ROW2 = 128   # layer 2:    [f2 0:64 | el2(f32) 64:66 | er2(f32) 66:68 | pad] = 256B


class Cfg:
    def __init__(self, n_nodes, n_cores, in_dim, n_hid, h_hid, n_cls, cnn_ch,
                 groups, kc):
        self.N = n_nodes
        self.C = n_cores
        self.NLOC = n_nodes // n_cores
        self.NBLK = (self.NLOC + P - 1) // P
        self.NPAD = self.NBLK * P
        self.IN_DIM = in_dim          # 512
        self.H = h_hid                # 4 heads
        self.D = n_hid                # 32 per head
        self.F = h_hid * n_hid        # 128
        self.NCLS = n_cls             # 64
        self.CNN_CH = cnn_ch          # 32
        # f-gather call plan: list of (start_chunk, n_chunks, table_base_row).
        # Each call's indices are int16 offsets from table_base_row, so a call
        # window covers 32768 rows; windows overlap, edges are assigned to
        # whichever call has room.
        self.GROUPS = groups
        self.K = sum(g[1] for g in groups)   # chunks per block
        self.KC = kc                  # CNN chunks per 64-node supertile (=32)
        self.KIN = in_dim // P
        self.ST = self.NPAD // 64
        self.NG = n_cores * self.NPAD
        self.NGH = self.NG // 2


def build_program(cfg: Cfg, use_act_lrelu: bool = True, max_stage: int = 99):
    c = cfg
    nc = bacc.Bacc("TRN2", target_bir_lowering=False, debug=False,
                   enable_asserts=False, num_devices=c.C, num_swdge_queues=4)

    def nextq():
        return 0

    CH3 = 3 * c.CNN_CH  # 96
    K, KC = c.K, c.KC

    # ---- external inputs ----
    xT_d = nc.dram_tensor("xT", [c.NBLK, P, c.KIN * P], F32,
                          kind="ExternalInput")
    wcat0_d = nc.dram_tensor("wcat0", [c.KIN, P, c.F + 8], F32, kind="ExternalInput")
    wcat1_d = nc.dram_tensor("wcat1", [P, c.F + 8], F32, kind="ExternalInput")
    wcat2_d = nc.dram_tensor("wcat2", [c.F, c.NCLS + 2], F32, kind="ExternalInput")
    eidx_d = nc.dram_tensor("eidx", [c.NBLK, P, K * 8], I16, kind="ExternalInput")
    edstv_d = nc.dram_tensor("edstv", [c.NBLK, P, K], F32, kind="ExternalInput")
    # transposed one-hot dst-selection bits: partition v, bit (k*P+p) set iff
    # edge slot (p,k) of the block has dst == v (used for the er broadcast)
    stbit_d = nc.dram_tensor("stbit", [c.NBLK, P, K * P // 16], I16,
                             kind="ExternalInput")
    bitc_d = nc.dram_tensor("bitc", [P, 16], I16, kind="ExternalInput")
    cidx_d = nc.dram_tensor("cidx", [c.ST, P, KC * 8], I16, kind="ExternalInput")
    cmsk_d = nc.dram_tensor("cmsk", [c.ST, P, 2 * KC], FP16, kind="ExternalInput")
    cw1ab_d = nc.dram_tensor("cw1ab", [P, 2 * CH3], FP16, kind="ExternalInput")
    cb1r_d = nc.dram_tensor("cb1r", [CH3, 1], F32, kind="ExternalInput")
    w2c_d = nc.dram_tensor("w2c", [CH3, 32], FP16, kind="ExternalInput")
    cb2_d = nc.dram_tensor("cb2", [P, 1], F32, kind="ExternalInput")

    y_d = nc.dram_tensor("y", [c.NPAD, c.NCLS], F32, kind="ExternalOutput")

    # ---- internal DRAM ----
    fe_loc = [nc.dram_tensor("fe_loc0", [c.NPAD, ROW], FP16),
              nc.dram_tensor("fe_loc1", [c.NPAD, ROW], FP16),
              nc.dram_tensor("fe_loc2", [c.NPAD, ROW2], FP16)]
    fe_full = [nc.dram_tensor("fe_full0", [c.NG, ROW], FP16, addr_space="Shared"),
               nc.dram_tensor("fe_full1", [c.NG, ROW], FP16, addr_space="Shared"),
               nc.dram_tensor("fe_full2", [c.NG, ROW2], FP16, addr_space="Shared")]
    lg_loc = nc.dram_tensor("lg_loc", [c.NPAD // 2, P], FP16)
    lg_full = nc.dram_tensor("lg_full", [c.NG // 2, P], FP16, addr_space="Shared")

    groups = [list(range(c.C))]
    h_tiles = {}

    with tile.TileContext(nc) as tc:
        with ExitStack() as ctx:
            consts = ctx.enter_context(tc.tile_pool(name="consts", bufs=1))
            hpool = ctx.enter_context(tc.tile_pool(name="hpool", bufs=1))

            nc.gpsimd.load_library(mlp)

            iota_t = consts.tile([P, P], F32)
            nc.gpsimd.iota(iota_t[:], pattern=[[1, P]], base=0, channel_multiplier=0,
                           allow_small_or_imprecise_dtypes=True)
            from concourse.masks import make_identity
            ident_t = consts.tile([P, P], F32)
            make_identity(nc, ident_t[:])
            wcat0_t = consts.tile([P, c.KIN * (c.F + 8)], F32)
            nc.sync.dma_start(out=wcat0_t[:].rearrange("p (k w) -> p k w", k=c.KIN),
                              in_=wcat0_d[:, :, :].transpose([1, 0, 2]))
            wcat1_t = consts.tile([P, c.F + 8], F32)
            nc.sync.dma_start(out=wcat1_t[:], in_=wcat1_d[:, :])
            wcat2_t = consts.tile([c.F, c.NCLS + 2], F32)
            nc.sync.dma_start(out=wcat2_t[:], in_=wcat2_d[:, :])
            cw1ab_t = consts.tile([P, 2 * CH3], FP16)
            nc.sync.dma_start(out=cw1ab_t[:], in_=cw1ab_d[:, :])
            cb1r_t = consts.tile([CH3, 1], F32)
            nc.sync.dma_start(out=cb1r_t[:], in_=cb1r_d[:, :])
            w2c_t = consts.tile([CH3, 32], FP16)
            nc.sync.dma_start(out=w2c_t[:], in_=w2c_d[:, :])
            cb2_t = consts.tile([P, 1], F32)
            nc.sync.dma_start(out=cb2_t[:], in_=cb2_d[:, :])
            bitc_t = consts.tile([P, 16], I16)
            nc.sync.dma_start(out=bitc_t[:], in_=bitc_d[:, :])

            def all_gather(src_t, dst_t):
                nc.gpsimd.collective_compute(
                    "AllGather", OP.bypass, replica_groups=groups,
                    ins=[src_t.ap().opt()], outs=[dst_t.ap().opt()])

            # chunked AllGather: the full tables are laid out CHUNK-MAJOR
            # (chunk, core, row) so each chunk's AG output is contiguous;
            # host-side row ids are remapped to match (see _chunk_map)
            def emit_ag(layer, t):
                for j0, nb, cum in _chunk_spans(c.NBLK, AGCH_FE):
                    if t == j0 + nb - 1:
                        src = fe_loc[layer][j0 * P:(j0 + nb) * P, :]
                        dst = fe_full[layer][cum * c.C * P:
                                             (cum + nb) * c.C * P, :]
                        nc.gpsimd.collective_compute(
                            "AllGather", OP.bypass, replica_groups=groups,
                            ins=[src.opt()], outs=[dst.opt()])

            with ExitStack() as gctx:
                sbA = gctx.enter_context(tc.tile_pool(name="sbA", bufs=2))
                psA = gctx.enter_context(tc.tile_pool(name="psA", bufs=2, space="PSUM"))
                sbC = gctx.enter_context(tc.tile_pool(name="sbC", bufs=2))
                psC = gctx.enter_context(tc.tile_pool(name="psC", bufs=2, space="PSUM"))

                def phase_a_block(layer, t):
                    fdim = c.F if layer < 2 else c.NCLS
                    wcols = fdim + 8 if layer < 2 else fdim + 2
                    if True:
                        pA = psA.tile([P, c.F + 8], F32, tag="pA", space="PSUM")
                        if layer == 0:
                            xk = sbA.tile([P, c.KIN * P], F32, tag="xk")
                            nc.sync.dma_start(out=xk[:], in_=xT_d[t, :, :])
                            for k in range(c.KIN):
                                nc.tensor.matmul(
                                    out=pA[:, 0:wcols],
                                    lhsT=xk[:, k * P:(k + 1) * P],
                                    rhs=wcat0_t[:, k * wcols:(k + 1) * wcols],
                                    start=(k == 0), stop=(k == c.KIN - 1))
                        else:
                            hin = h_tiles[(layer, t)]
                            pT = psA.tile([P, P], F32, tag="pT", space="PSUM")
                            nc.tensor.transpose(out=pT[:], in_=hin[:],
                                                identity=ident_t[:])
                            hT = sbA.tile([P, P], F32, tag="hT")
                            nc.vector.tensor_copy(out=hT[:], in_=pT[:])
                            nc.tensor.matmul(
                                out=pA[:, 0:wcols], lhsT=hT[:],
                                rhs=(wcat1_t[:] if layer == 1 else wcat2_t[:]),
                                start=True, stop=True)
                        # keep er for this block resident in SBUF (fp16): the
                        # edge phase broadcasts it to edge slots via matmul
                        nh2 = c.H if layer < 2 else 1
                        ersb = hpool.tile([P, 8], FP16, tag=f"er{layer}_{t}")
                        nc.vector.tensor_copy(
                            out=ersb[:, 0:nh2],
                            in_=pA[:, fdim + nh2:fdim + 2 * nh2])
                        h_tiles[(f"er{layer}", t)] = ersb
                        fdim_ = fdim
                        nsl = fdim_ + 2 * (wcols - fdim_)  # fp16 slots used
                        fea = sbA.tile([P, c.F + 16], FP16, tag="fea")
                        nc.vector.tensor_copy(out=fea[:, 0:fdim_], in_=pA[:, 0:fdim_])
                        nc.vector.tensor_copy(
                            out=fea[:, fdim_:nsl].bitcast(F32),
                            in_=pA[:, fdim_:wcols])
                        nc.sync.dma_start(out=fe_loc[layer][t * P:(t + 1) * P, 0:nsl],
                                          in_=fea[:, 0:nsl])

                def phase_c(layer, after_block=None):
                    fdim = c.F if layer < 2 else c.NCLS      # 128 / 64
                    nh = c.H if layer < 2 else 1
                    rb = ROW if layer < 2 else ROW2
                    ecol = fdim  # el col offset within row
                    KP16 = K * P // 16
                    for b in range(c.NBLK):
                        idx = sbC.tile([P, K * 8], I16, tag="idx")
                        nc.sync.dma_start(out=idx[:], in_=eidx_d[b, :, :])
                        dstv = sbC.tile([P, K], F32, tag="dstv")
                        nc.sync.dma_start(out=dstv[:], in_=edstv_d[b, :, :])
                        stb = sbC.tile([P, KP16], I16, tag="stb")
                        nc.sync.dma_start(out=stb[:], in_=stbit_d[b, :, :])
                        feg = sbC.tile([P, K * ROW], FP16, tag="feg")
                        feg3 = feg[:, 0:K * rb].rearrange("p (k r) -> p k r", r=rb)
                        for gs, gnk, gbase in c.GROUPS:
                            nc.gpsimd.dma_gather(
                                feg3[:, gs:gs + gnk, :],
                                fe_full[layer][gbase:c.NG, :],
                                idx[:, gs * 8:(gs + gnk) * 8],
                                gnk * P, gnk * P, rb, queue_num=nextq())
                        # unpack transposed one-hot St[v, (k,p)] from bits
                        andt = sbC.tile([P, K * P], I16, tag="andt")
                        nc.vector.tensor_tensor(
                            out=andt[:].rearrange("p (j e) -> p j e", e=16),
                            in0=stb[:].unsqueeze(2).to_broadcast([P, KP16, 16]),
                            in1=bitc_t[:].unsqueeze(1).to_broadcast([P, KP16, 16]),
                            op=OP.bitwise_and)
                        sth = sbC.tile([P, K * P], FP16, tag="sth")
                        nc.vector.tensor_tensor(
                            out=sth[:].rearrange("p (j e) -> p j e", e=16),
                            in0=andt[:].rearrange("p (j e) -> p j e", e=16),
                            in1=bitc_t[:].unsqueeze(1).to_broadcast([P, KP16, 16]),
                            op=OP.is_equal)
                        # er_edge[p, (k,h)] = er_blk[dstv[p,k], h] via K matmuls
                        ersb = h_tiles[(f"er{layer}", b)]
                        erp = psC.tile([P, K * c.H], F32, tag="erp", space="PSUM")
                        for k in range(K):
                            nc.tensor.matmul(
                                out=erp[:, k * nh:k * nh + nh],
                                lhsT=sth[:, k * P:(k + 1) * P],
                                rhs=ersb[:, 0:nh], start=True, stop=True)
                        # f32 view of el (in feg)
                        el_f32 = feg3[:, :, fdim:fdim + 2 * nh].bitcast(F32)
                        # S[p, k, v] = (v == dstv[p, k])
                        s_all = sbC.tile([P, K * P], FP16, tag="sall")
                        nc.vector.tensor_tensor(
                            out=s_all[:].rearrange("p (k v) -> p k v", v=P),
                            in0=iota_t[:].unsqueeze(1).to_broadcast([P, K, P]),
                            in1=dstv[:].unsqueeze(2).to_broadcast([P, K, P]),
                            op=OP.is_equal)
                        # e = lrelu(el + er); ex = exp(e) overwrites el cols in feg
                        e_t = sbC.tile([P, K * c.H], F32, tag="e_t")
                        e3 = e_t[:, 0:K * nh].rearrange("p (k h) -> p k h", h=nh)
                        nc.vector.tensor_tensor(
                            out=e3, in0=el_f32,
                            in1=erp[:, 0:K * nh].rearrange("p (k h) -> p k h", h=nh),
                            op=OP.add)
                        nc.vector.scalar_tensor_tensor(
                            out=e_t[:, 0:K * nh], in0=e_t[:, 0:K * nh], scalar=GAT_NEG,
                            in1=e_t[:, 0:K * nh], op0=OP.mult, op1=OP.max)
                        nc.scalar.activation(out=feg3[:, :, ecol:ecol + nh], in_=e3,
                                             func=AF.Exp)
                        # exe = exp(e) pre-expanded along D on the (idle) scalar
                        # engine so the g-multiply is a contiguous fp16 DVE op
                        dph = c.D if layer < 2 else fdim
                        exe = sbC.tile([P, K * c.F], FP16, tag="exe")
                        exe4 = exe[:, 0:K * fdim].rearrange(
                            "p (k h d) -> p k h d", k=K, d=dph)
                        nc.scalar.activation(
                            out=exe4,
                            in_=e3.unsqueeze(3).to_broadcast([P, K, nh, dph]),
                            func=AF.Exp)
                        # g = f * ex, in place
                        nc.vector.tensor_tensor(
                            out=feg3[:, :, 0:fdim], in0=feg3[:, :, 0:fdim],
                            in1=exe[:, 0:K * fdim].rearrange(
                                "p (k d) -> p k d", k=K),
                            op=OP.mult)
                        gw = fdim + nh
                        agg = psC.tile([P, c.F + c.H], F32, tag="agg", space="PSUM")
                        for k in range(K):
                            nc.tensor.matmul(
                                out=agg[:, 0:gw],
                                lhsT=s_all[:, k * P:(k + 1) * P],
                                rhs=feg[:, k * rb:k * rb + gw],
                                start=(k == 0), stop=(k == K - 1))
                        rec = sbC.tile([P, c.H], F32, tag="rec")
                        nc.vector.tensor_scalar_max(rec[:, 0:nh],
                                                    agg[:, fdim:fdim + nh], 1e-30)
                        nc.vector.reciprocal(out=rec[:, 0:nh], in_=rec[:, 0:nh])
                        if layer < 2:
                            z = sbC.tile([P, c.F], F32, tag="z")
                            for h in range(nh):
                                nc.vector.tensor_scalar_mul(
                                    z[:, h * c.D:(h + 1) * c.D],
                                    agg[:, h * c.D:(h + 1) * c.D], rec[:, h:h + 1])
                            zm = sbC.tile([P, c.F], F32, tag="zm")
                            nc.vector.tensor_scalar_min(zm[:], z[:], 0.0)
                            ze = sbC.tile([P, c.F], F32, tag="ze")
                            nc.scalar.activation(out=ze[:], in_=zm[:], func=AF.Exp)
                            nc.vector.tensor_scalar_max(z[:], z[:], 0.0)
                            hout = hpool.tile([P, c.F], F32, tag=f"h{layer + 1}_{b}")
                            nc.vector.scalar_tensor_tensor(
                                out=hout[:], in0=ze[:], scalar=-1.0, in1=z[:],
                                op0=OP.add, op1=OP.add)
                            h_tiles[(layer + 1, b)] = hout
                        else:
                            lgt = sbC.tile([P, c.NCLS], F32, tag="lgt")
                            nc.vector.tensor_scalar_mul(lgt[:], agg[:, 0:c.NCLS],
                                                        rec[:, 0:1])
                            lgb = sbC.tile([P, c.NCLS], FP16, tag="lgb")
                            nc.vector.tensor_copy(out=lgb[:], in_=lgt[:])
                            nc.sync.dma_start(
                                out=lg_loc[b * 64:(b + 1) * 64, :]
                                    .rearrange("r (a d) -> (r a) d", a=2),
                                in_=lgb[:])
                        if after_block is not None:
                            after_block(b)

                def next_layer_cb(nl):
                    def cb(b):
                        phase_a_block(nl, b)
                        emit_ag(nl, b)
                    return cb

                def lg_cb(b):
                    # chunked AllGather of the logit table (chunk-major)
                    for j0, nb, cum in _chunk_spans(c.NBLK, AGCH_LG):
                        if b == j0 + nb - 1:
                            src = lg_loc[j0 * 64:(j0 + nb) * 64, :]
                            dst = lg_full[cum * c.C * 64:
                                          (cum + nb) * c.C * 64, :]
                            nc.gpsimd.collective_compute(
                                "AllGather", OP.bypass,
                                replica_groups=groups,
                                ins=[src.opt()], outs=[dst.opt()])

                if max_stage >= 1:
                    for t in range(c.NBLK):
                        phase_a_block(0, t)
                        emit_ag(0, t)
                if max_stage >= 2:
                    phase_c(0, after_block=next_layer_cb(1))
                if max_stage >= 4:
                    phase_c(1, after_block=next_layer_cb(2))
                if max_stage >= 6:
                    phase_c(2, after_block=lg_cb)

            # ---------------- CNN head ----------------
            with ExitStack() as dctx:
                sbD = dctx.enter_context(tc.tile_pool(name="sbD", bufs=2))
                sbD3 = dctx.enter_context(tc.tile_pool(name="sbD3", bufs=3))
                psD = dctx.enter_context(tc.tile_pool(name="psD", bufs=2, space="PSUM"))

                for st in range(c.ST if max_stage >= 8 else 0):
                    cidx = sbD.tile([P, KC * 8], I16, tag="cidx")
                    nc.sync.dma_start(out=cidx[:], in_=cidx_d[st, :, :])
                    cmsk = sbD.tile([P, 2 * KC], FP16, tag="cmsk")
                    nc.sync.dma_start(out=cmsk[:], in_=cmsk_d[st, :, :])
                    for g in range(KC // 8 if max_stage >= 9 else 0):
                        o = g * 8
                        # per-group gather + select so group g's compute
                        # overlaps group g+1's gather DMA
                        Xg = sbD3.tile([P, 8 * P], FP16, tag=f"X{g}")
                        X3 = Xg[:].rearrange("p (k r) -> p k r", r=P)
                        nc.gpsimd.dma_gather(
                            X3[:, 0:8, :], lg_full[:, :],
                            cidx[:, o * 8:(o + 8) * 8], 8 * P, 8 * P, P,
                            queue_num=nextq())
                        X64 = sbD3.tile([P, 8 * 64], FP16, tag=f"X64{g}")
                        X64_3 = X64[:].rearrange("p (k r) -> p k r", r=64)
                        Xo = sbD3.tile([P, 8 * 64], FP16, tag=f"Xo{g}")
                        Xo_3 = Xo[:].rearrange("p (k r) -> p k r", r=64)
                        # exact parity select: X64 = Xl*(1-m) + Xr*m (masks 0/1)
                        nc.vector.tensor_tensor(
                            out=X64_3, in0=X3[:, :, 0:64],
                            in1=cmsk[:, o:o + 8].unsqueeze(2)
                                .to_broadcast([P, 8, 64]),
                            op=OP.mult)
                        nc.vector.tensor_tensor(
                            out=Xo_3, in0=X3[:, :, 64:P],
                            in1=cmsk[:, KC + o:KC + o + 8].unsqueeze(2)
                                .to_broadcast([P, 8, 64]),
                            op=OP.mult)
                        nc.vector.tensor_tensor(out=X64[:], in0=X64[:],
                                                in1=Xo[:], op=OP.add)
                        y2p = psD.tile([64, 512], F32, tag="y2p", space="PSUM")
                        for par in range(2):
                            y1p = psD.tile([CH3, 512], F32, tag=f"y1p{par}",
                                           space="PSUM")
                            nc.tensor.matmul(
                                out=y1p[:],
                                lhsT=cw1ab_t[par * 64:(par + 1) * 64,
                                             par * CH3:(par + 1) * CH3],
                                rhs=X64[par * 64:(par + 1) * 64, :],
                                start=True, stop=True)
                            y1s = sbD3.tile([CH3, 512], FP16, tag=f"y1s{par}")
                            if use_act_lrelu:
                                nc.scalar.activation(out=y1s[:], in_=y1p[:],
                                                     func=AF.Lrelu,
                                                     bias=cb1r_t[:, 0:1],
                                                     alpha=CNN_NEG)
                            else:
                                yt = sbD3.tile([CH3, 512], F32, tag=f"yt{par}")
                                nc.vector.tensor_scalar_add(yt[:], y1p[:],
                                                            cb1r_t[:, 0:1])
                                nc.vector.scalar_tensor_tensor(
                                    out=y1s[:], in0=yt[:], scalar=CNN_NEG, in1=yt[:],
                                    op0=OP.mult, op1=OP.max)
                            nc.tensor.matmul(out=y2p[par * 32:(par + 1) * 32, :],
                                             lhsT=w2c_t[:], rhs=y1s[:],
                                             start=True, stop=True,
                                             tile_position=(0, par * 32))
                        ysb = sbD.tile([64, 512], F32, tag="ysb")
                        if use_act_lrelu:
                            nc.scalar.activation(out=ysb[:], in_=y2p[:], func=AF.Lrelu,
                                                 bias=cb2_t[0:64, 0:1], alpha=CNN_NEG)
                        else:
                            nc.vector.tensor_scalar_add(ysb[:], y2p[:],
                                                        cb2_t[0:64, 0:1])
                            nc.vector.scalar_tensor_tensor(
                                out=ysb[:], in0=ysb[:], scalar=CNN_NEG, in1=ysb[:],
                                op0=OP.mult, op1=OP.max)
                        r0 = st * 64 + g * 16
                        nc.sync.dma_start(
                            out=y_d[r0:r0 + 16, :].rearrange("(b a) d -> a b d", a=2),
                            in_=ysb[0:64:32, :].rearrange("a (b d) -> a b d", d=c.NCLS))
    # Align SWDGE queue assignment with Tile's DMASW sem-lane rotation:
    # sem lane s must always service the same queue, so queue = lane % 4.
    from concourse.tile_sem_assignment import PROC_NAME_TO_IDX
    lane_to_q = {}
    for q in range(8):
        if f"DMASW{q}" in PROC_NAME_TO_IDX:
            lane_to_q[PROC_NAME_TO_IDX[f"DMASW{q}"]] = q % 4
    nfix = 0
    for blk in nc.m.functions[0].blocks:
        for inst in blk.instructions:
            if isinstance(inst, mybir.InstDMAGatherAnt):
                proc = inst.bass_scheduled_proc
                if proc in lane_to_q:
                    inst.queue_num = lane_to_q[proc]
                    nfix += 1
    nc.compile()
    return nc


# ------------------------------------------------------------------
# host-side preparation
# ------------------------------------------------------------------

def _block_diag_a(a):
    a = np.asarray(a, np.float32)
    hh, dd = a.shape
    out = np.zeros((hh * dd, hh), np.float32)
    for h in range(hh):
        out[h * dd:(h + 1) * dd, h] = a[h]
    return out


def _wrap16(slots, ncols):
    """slots: int array [NI] -> [128, NI//16] int16, wrapped in 16 partitions
    and replicated into all 8 16-partition groups."""
    ni = slots.shape[0]
    m = np.zeros((P, ncols), np.int16)
    j = np.arange(ni)
    for rep in range(8):
        m[j % 16 + 16 * rep, j // 16] = slots.astype(np.int16)
    return m


def compute_groups(nloc, dst, ncores):
    """f-gather call plan from the worst per-block edge count.

    Calls are capped at 1024 indices (8 chunks) by the gather ucode ring.
    Call 0 addresses table rows [0, 32768), call 1 rows [NG-32768, NG);
    overflow calls address [0, 32768) again (they only ever hold edges
    whose src row < 32768).
    """
    nblk = (nloc + P - 1) // P
    worst = 0
    for core in range(ncores):
        lo = core * nloc
        m = (dst >= lo) & (dst < lo + nloc)
        cnt = np.bincount((dst[m] - lo) // P, minlength=nblk)
        worst = max(worst, int(cnt.max()))
    npad = nblk * P
    base2 = ncores * npad - 32768
    if worst <= 1024:
        return [(0, (worst + P - 1) // P, 0)], base2
    groups = [(0, 8, 0), (8, 8, base2)]
    left = worst - 2048
    gs = 16
    while left > 0:
        gnk = min(8, (left + P - 1) // P)
        groups.append((gs, gnk, 0))
        gs += gnk
        left -= gnk * P
    return groups, base2


def prepare_inputs(cfg, x, src, dst, paths, W0, al0, ar0, W1, al1, ar1,
                   W2, al2, ar2, cw1, cb1, cw2, cb2):
    c = cfg
    x = np.asarray(x, np.float32)
    src = np.asarray(src, np.int64)
    dst = np.asarray(dst, np.int64)
    paths = np.asarray(paths, np.int64)
    W0 = np.asarray(W0, np.float32)
    W1 = np.asarray(W1, np.float32)
    W2 = np.asarray(W2, np.float32)
    al2 = np.asarray(al2, np.float32)
    ar2 = np.asarray(ar2, np.float32)
    cw1 = np.asarray(cw1, np.float32)
    cw2 = np.asarray(cw2, np.float32)

    wcat0 = np.concatenate(
        [W0, W0 @ _block_diag_a(al0), W0 @ _block_diag_a(ar0)], axis=1
    ).astype(np.float32).reshape(c.KIN, P, c.F + 8)
    wcat1 = np.ascontiguousarray(np.concatenate(
        [W1, W1 @ _block_diag_a(al1), W1 @ _block_diag_a(ar1)], axis=1
    ).astype(np.float32))
    wcat2 = np.ascontiguousarray(np.concatenate(
        [W2, W2 @ al2.reshape(-1, 1), W2 @ ar2.reshape(-1, 1)], axis=1
    ).astype(np.float32))

    ch = c.CNN_CH
    CH3 = 3 * ch
    cw1e = np.zeros((50, CH3), np.float32)
    for co in range(ch):
        for pp in range(3):
            for ci in range(10):
                for t in range(3):
                    cw1e[ci * 5 + pp + t, co * 3 + pp] = cw1[co, ci, t, 0]
    cw1ab = np.zeros((P, 2 * CH3), np.float32)
    cw1ab[0:50, 0:CH3] = cw1e
    cw1ab[64:114, CH3:2 * CH3] = cw1e
    cb1r = np.repeat(np.asarray(cb1, np.float32), 3).reshape(CH3, 1)
    w2c = np.zeros((CH3, 32), np.float32)
    for co in range(ch):
        for pp in range(3):
            w2c[co * 3 + pp, :] = cw2[0, co, pp, 0]
    cb2a = np.full((P, 1), np.float32(np.asarray(cb2).reshape(-1)[0]))

    grow = _chunk_map(c.NBLK, AGCH_FE, c.NLOC, c.C, P)
    lgmap = _chunk_map(c.NBLK, AGCH_LG, c.NLOC, c.C, 64)

    in_maps = []
    for core in range(c.C):
        lo, hi = core * c.NLOC, (core + 1) * c.NLOC
        xs = np.zeros((c.NPAD, c.IN_DIM), np.float32)
        xs[:c.NLOC] = x[lo:hi]
        # per-block [q, (k, n)] layout so each block loads as one
        # contiguous 128x2KB DMA
        xT = np.ascontiguousarray(
            xs.reshape(c.NBLK, P, c.KIN, P).transpose(0, 3, 2, 1)
        ).reshape(c.NBLK, P, c.KIN * P)

        m = (dst >= lo) & (dst < hi)
        es, ed = grow(src[m]), dst[m] - lo
        order = np.argsort(ed, kind="stable")
        es, ed = es[order], ed[order]
        eblk = ed // P
        eidx = np.zeros((c.NBLK, P, c.K * 8), np.int16)
        edstv = np.full((c.NBLK, P, c.K), -1.0, np.float32)
        stbit = np.zeros((c.NBLK, P, c.K * P // 16), np.uint16)
        base2 = c.NG - 32768  # second-window base; window = [base2, NG)
        for b in range(c.NBLK):
            bm = eblk == b
            bs, bd = es[bm], ed[bm]
            # assign edges to gather calls: call 0 covers rows [0, 32768),
            # call 1 covers [base2, NG), extra calls cover [0, 32768).
            caps = [gnk * P for _, gnk, _ in c.GROUPS]
            if len(c.GROUPS) == 1:
                assert len(bs) <= caps[0]
                asn = [np.arange(len(bs))]
            else:
                i1 = np.where(bs >= 32768)[0]       # must use window-1 call
                i0 = np.where(bs < base2)[0]        # must use a base-0 call
                iflex = np.where((bs >= base2) & (bs < 32768))[0]
                assert len(i1) <= caps[1], (core, b, len(i1))
                nf2 = min(caps[1] - len(i1), len(iflex))
                asn = [None] * len(c.GROUPS)
                asn[1] = np.concatenate([i1, iflex[:nf2]])
                rest = np.concatenate([i0, iflex[nf2:]]).astype(np.int64)
                asn[0] = rest[:caps[0]]
                rest = rest[caps[0]:]
                for gi in range(2, len(c.GROUPS)):
                    asn[gi] = rest[:caps[gi]]
                    rest = rest[caps[gi]:]
                assert rest.size == 0, (core, b, rest.size)
            for gi, (gstart, gnk, gbase) in enumerate(c.GROUPS):
                sel = asn[gi]
                hs, hd = bs[sel], bd[sel]
                ne = hs.shape[0]
                # pad slots point at spread-out (valid) rows: same-address
                # row-0 fetches serialize on one HBM bank and skew the DMA
                # engine load badly
                g_idx = (np.arange(gnk * P, dtype=np.int64) * 997) % 16384
                g_idx[:ne] = hs - gbase
                assert ne == 0 or (g_idx[:ne].min() >= 0
                                   and g_idx[:ne].max() < 32768)
                j = np.arange(ne)
                # slot j of this call -> chunk gstart + j//128, partition j%128
                hdr = (hd - b * P).astype(np.int64)
                edstv[b, j % P, gstart + j // P] = hdr.astype(np.float32)
                slot = (gstart + j // P) * P + j % P
                np.bitwise_or.at(
                    stbit[b], (hdr, slot >> 4),
                    (np.uint16(1) << (slot & 15).astype(np.uint16)))
                eidx[b, :, gstart * 8:(gstart + gnk) * 8] = _wrap16(
                    g_idx, gnk * 8)


        pl = np.zeros((c.NPAD, 50), np.int64)
        pl[:c.NLOC] = paths[lo:hi].reshape(c.NLOC, 50)
        pgp = lgmap(pl)                                      # pair row
        ppar = (pl % c.NLOC) & 1                             # node parity
        cidx = np.zeros((c.ST, P, c.KC * 8), np.int16)
        cmsk = np.zeros((c.ST, P, 2 * c.KC), np.float16)
        for st in range(c.ST):
            j = np.arange(c.KC * P)
            k, p = j // P, j % P
            n = st * 64 + 2 * k + (p >= 64)
            q = p % 64
            valid = q < 50
            qm = np.minimum(q, 49)
            slots = np.where(valid, pgp[n, qm], (j * 1997) % (c.NG // 2))
            par = np.where(valid, ppar[n, qm], 0)
            cidx[st] = _wrap16(slots, c.KC * 8)
            cmsk[st, p, k] = (1 - par).astype(np.float16)
            cmsk[st, p, c.KC + k] = par.astype(np.float16)
        in_maps.append({
            "xT": xT, "wcat0": wcat0, "wcat1": wcat1, "wcat2": wcat2,
            "eidx": eidx, "edstv": edstv, "stbit": stbit.view(np.int16),
            "bitc": np.broadcast_to(
                (np.uint16(1) << np.arange(16, dtype=np.uint16))
                .view(np.int16), (P, 16)).copy(),
            "cidx": cidx, "cmsk": cmsk,
            "cw1ab": cw1ab.astype(np.float16), "cb1r": cb1r,
            "w2c": w2c.astype(np.float16), "cb2": cb2a,
        })
    return in_maps


_CACHE = {}


def kernel(**inputs):
    x = np.asarray(inputs["x"], np.float32)
    n_nodes = x.shape[0]
    dst = np.asarray(inputs["dst"], np.int64)
    ncores = 8
    nloc = n_nodes // ncores
    groups, _ = compute_groups(nloc, dst, ncores)
    cfg = Cfg(n_nodes=n_nodes, n_cores=ncores, in_dim=x.shape[1],
              n_hid=32, h_hid=4, n_cls=64,
              cnn_ch=int(np.asarray(inputs["cw1"]).shape[0]),
              groups=groups, kc=32)

    key = (n_nodes, cfg.IN_DIM, tuple(groups))
    if key not in _CACHE:
        _CACHE[key] = build_program(cfg)
    nc = _CACHE[key]

    in_maps = prepare_inputs(cfg, **inputs)
    res = bass_utils.run_bass_kernel_spmd(nc, in_maps, core_ids=list(range(cfg.C)))
    y = np.concatenate(
        [res.results[core]["y"][:cfg.NLOC] for core in range(cfg.C)], axis=0)
    return np.ascontiguousarray(y.astype(np.float32))



# revision 50
# speedup vs baseline: 1.2474x; 1.0770x over previous
"""Trainium2 Bass kernel for the 3-layer GAT + path-CNN model (nn_GAT_41729902248227).

Node-sharded graph parallelism over 8 NeuronCores:
 - Nodes sharded contiguously (N/8 per core, padded to 49 blocks of 128).
 - Edges sharded by dst, sorted by dst, grouped per 128-dst-node block.
   Within a block, edges are split by src table half (int16 gather index
   limit) and padded to a uniform per-half chunk count (KS) so a single SPMD
   program serves all cores.
 - Per GAT layer: a dense phase computes [f | el | er] = h @ [W | W@Al | W@Ar]
   per 128-node tile (one matmul) and writes 768B rows to a DRAM table that is
   AllGathered. The edge phase uses the dma_gather Q7 ucode: per block, two
   row gathers fetch f/el by src (lo/hi table halves) and one narrow-column
   gather fetches er by dst from the LOCAL table (dst indices are core-local
   so they fit int16 without splitting). One-hot dst-selection matrices are
   built on-device (iota + is_equal) and the edge softmax + aggregation folds
   into PE matmuls accumulating [sum ex*f | sum ex] in PSUM, followed by
   per-head normalization and ELU.
 - CNN head: logits stored bf16 pair-packed (two nodes per 256B row) so the
   row count fits int16; one dma_gather per 64-node supertile fetches all path
   rows, a predicated copy selects the node half, and conv1/conv2 run as bf16
   matmuls with host-expanded block weights using PE row/col tile grouping.
"""

import sys

sys.path.insert(0, "/opt/trn_rl_repo")

from contextlib import ExitStack

import ml_dtypes
import numpy as np

import concourse.bacc as bacc
import concourse.bass as bass
import concourse.mybir as mybir
import concourse.tile as tile
from concourse import bass_utils
from concourse.library_config import mlp

F32 = mybir.dt.float32
BF16 = mybir.dt.bfloat16
FP16 = mybir.dt.float16
I16 = mybir.dt.int16
AF = mybir.ActivationFunctionType
OP = mybir.AluOpType
NPBF16 = ml_dtypes.bfloat16

P = 128
GAT_NEG = 0.2
CNN_NEG = 0.01
AGCH_FE = 8    # blocks per fe AllGather chunk
AGCH_LG = 16   # blocks per lg AllGather chunk


def _chunk_spans(nblk, agch):
    """[(first_block, n_blocks, cumulative_blocks_before), ...]"""
    out = []
    base = 0
    for j0 in range(0, nblk, agch):
        nb = min(agch, nblk - j0)
        out.append((j0, nb, base))
        base += nb
    return out


def _chunk_map(nblk, agch, nloc, ncores, rows_per_block):
    """Vectorized global-node-id -> chunk-major full-table row id."""
    spans = _chunk_spans(nblk, agch)
    nch = len(spans)
    j0s = np.zeros(nch, np.int64)
    nbs = np.zeros(nch, np.int64)
    cums = np.zeros(nch, np.int64)
    for i, (j0, nb, cum) in enumerate(spans):
        j0s[i], nbs[i], cums[i] = j0, nb, cum
    rb = rows_per_block

    def f(v):
        cc, r = v // nloc, v % nloc
        j = (r // P) // agch
        off = (r - j0s[j] * P) * rb // P
        return cums[j] * ncores * rb + cc * nbs[j] * rb + off

    return f
# fe table rows are fp16 slots; el/er stay f32, bit-packed into pairs of slots.
ROW = 256    # layers 0/1: [f 0:128 | el(f32) 128:136 | er(f32) 136:144 | pad] = 512B
ROW2 = 128   # layer 2:    [f2 0:64 | el2(f32) 64:66 | er2(f32) 66:68 | pad] = 256B


class Cfg:
    def __init__(self, n_nodes, n_cores, in_dim, n_hid, h_hid, n_cls, cnn_ch,
                 groups, kc):
        self.N = n_nodes
        self.C = n_cores
        self.NLOC = n_nodes // n_cores
        self.NBLK = (self.NLOC + P - 1) // P
        self.NPAD = self.NBLK * P
        self.IN_DIM = in_dim          # 512
        self.H = h_hid                # 4 heads
        self.D = n_hid                # 32 per head
        self.F = h_hid * n_hid        # 128
        self.NCLS = n_cls             # 64
        self.CNN_CH = cnn_ch          # 32
        # f-gather call plan: list of (start_chunk, n_chunks, table_base_row).
        # Each call's indices are int16 offsets from table_base_row, so a call
        # window covers 32768 rows; windows overlap, edges are assigned to
        # whichever call has room.
        self.GROUPS = groups
        self.K = sum(g[1] for g in groups)   # chunks per block
        self.KC = kc                  # CNN chunks per 64-node supertile (=32)
        self.KIN = in_dim // P
        self.ST = self.NPAD // 64
        self.NG = n_cores * self.NPAD
        self.NGH = self.NG // 2


def build_program(cfg: Cfg, use_act_lrelu: bool = True, max_stage: int = 99):
    c = cfg
    nc = bacc.Bacc("TRN2", target_bir_lowering=False, debug=False,
                   enable_asserts=False, num_devices=c.C, num_swdge_queues=4)

    def nextq():
        return 0

    CH3 = 3 * c.CNN_CH  # 96
    K, KC = c.K, c.KC

    # ---- external inputs ----
    xT_d = nc.dram_tensor("xT", [c.NBLK, P, c.KIN * P], F32,
                          kind="ExternalInput")
    wcat0_d = nc.dram_tensor("wcat0", [c.KIN, P, c.F + 8], F32, kind="ExternalInput")
    wcat1_d = nc.dram_tensor("wcat1", [P, c.F + 8], F32, kind="ExternalInput")
    wcat2_d = nc.dram_tensor("wcat2", [c.F, c.NCLS + 2], F32, kind="ExternalInput")
    eidx_d = nc.dram_tensor("eidx", [c.NBLK, P, K * 8], I16, kind="ExternalInput")
    edstv_d = nc.dram_tensor("edstv", [c.NBLK, P, K], F32, kind="ExternalInput")
    # transposed one-hot dst-selection bits: partition v, bit (k*P+p) set iff
    # edge slot (p,k) of the block has dst == v (used for the er broadcast)
    stbit_d = nc.dram_tensor("stbit", [c.NBLK, P, K * P // 16], I16,
                             kind="ExternalInput")
    bitc_d = nc.dram_tensor("bitc", [P, 16], I16, kind="ExternalInput")
    cidx_d = nc.dram_tensor("cidx", [c.ST, P, KC * 8], I16, kind="ExternalInput")
    cmsk_d = nc.dram_tensor("cmsk", [c.ST, P, 2 * KC], FP16, kind="ExternalInput")
    cw1ab_d = nc.dram_tensor("cw1ab", [P, 2 * CH3], FP16, kind="ExternalInput")
    cb1r_d = nc.dram_tensor("cb1r", [CH3, 1], F32, kind="ExternalInput")
    w2c_d = nc.dram_tensor("w2c", [CH3, 32], FP16, kind="ExternalInput")
    cb2_d = nc.dram_tensor("cb2", [P, 1], F32, kind="ExternalInput")

    y_d = nc.dram_tensor("y", [c.NPAD, c.NCLS], F32, kind="ExternalOutput")

    # ---- internal DRAM ----
    fe_loc = [nc.dram_tensor("fe_loc0", [c.NPAD, ROW], FP16),
              nc.dram_tensor("fe_loc1", [c.NPAD, ROW], FP16),
              nc.dram_tensor("fe_loc2", [c.NPAD, ROW2], FP16)]
    fe_full = [nc.dram_tensor("fe_full0", [c.NG, ROW], FP16, addr_space="Shared"),
               nc.dram_tensor("fe_full1", [c.NG, ROW], FP16, addr_space="Shared"),
               nc.dram_tensor("fe_full2", [c.NG, ROW2], FP16, addr_space="Shared")]
    lg_loc = nc.dram_tensor("lg_loc", [c.NPAD // 2, P], FP16)
    lg_full = nc.dram_tensor("lg_full", [c.NG // 2, P], FP16, addr_space="Shared")

    groups = [list(range(c.C))]
    h_tiles = {}

    with tile.TileContext(nc) as tc:
        with ExitStack() as ctx:
            consts = ctx.enter_context(tc.tile_pool(name="consts", bufs=1))
            hpool = ctx.enter_context(tc.tile_pool(name="hpool", bufs=1))

            nc.gpsimd.load_library(mlp)

            iota_t = consts.tile([P, P], F32)
            nc.gpsimd.iota(iota_t[:], pattern=[[1, P]], base=0, channel_multiplier=0,
                           allow_small_or_imprecise_dtypes=True)
            from concourse.masks import make_identity
            ident_t = consts.tile([P, P], F32)
            make_identity(nc, ident_t[:])
            wcat0_t = consts.tile([P, c.KIN * (c.F + 8)], F32)
            nc.sync.dma_start(out=wcat0_t[:].rearrange("p (k w) -> p k w", k=c.KIN),
                              in_=wcat0_d[:, :, :].transpose([1, 0, 2]))
            wcat1_t = consts.tile([P, c.F + 8], F32)
            nc.sync.dma_start(out=wcat1_t[:], in_=wcat1_d[:, :])
            wcat2_t = consts.tile([c.F, c.NCLS + 2], F32)
            nc.sync.dma_start(out=wcat2_t[:], in_=wcat2_d[:, :])
            cw1ab_t = consts.tile([P, 2 * CH3], FP16)
            nc.sync.dma_start(out=cw1ab_t[:], in_=cw1ab_d[:, :])
            cb1r_t = consts.tile([CH3, 1], F32)
            nc.sync.dma_start(out=cb1r_t[:], in_=cb1r_d[:, :])
            w2c_t = consts.tile([CH3, 32], FP16)
            nc.sync.dma_start(out=w2c_t[:], in_=w2c_d[:, :])
            cb2_t = consts.tile([P, 1], F32)
            nc.sync.dma_start(out=cb2_t[:], in_=cb2_d[:, :])
            bitc_t = consts.tile([P, 16], I16)
            nc.sync.dma_start(out=bitc_t[:], in_=bitc_d[:, :])

            def all_gather(src_t, dst_t):
                nc.gpsimd.collective_compute(
                    "AllGather", OP.bypass, replica_groups=groups,
                    ins=[src_t.ap().opt()], outs=[dst_t.ap().opt()])

            # chunked AllGather: the full tables are laid out CHUNK-MAJOR
            # (chunk, core, row) so each chunk's AG output is contiguous;
            # host-side row ids are remapped to match (see _chunk_map)
            def emit_ag(layer, t):
                for j0, nb, cum in _chunk_spans(c.NBLK, AGCH_FE):
                    if t == j0 + nb - 1:
                        src = fe_loc[layer][j0 * P:(j0 + nb) * P, :]
                        dst = fe_full[layer][cum * c.C * P:
                                             (cum + nb) * c.C * P, :]
                        nc.gpsimd.collective_compute(
                            "AllGather", OP.bypass, replica_groups=groups,
                            ins=[src.opt()], outs=[dst.opt()])

            with ExitStack() as gctx:
                sbA = gctx.enter_context(tc.tile_pool(name="sbA", bufs=2))
                psA = gctx.enter_context(tc.tile_pool(name="psA", bufs=2, space="PSUM"))
                sbC = gctx.enter_context(tc.tile_pool(name="sbC", bufs=3))
                psC = gctx.enter_context(tc.tile_pool(name="psC", bufs=2, space="PSUM"))

                def phase_a_block(layer, t):
                    fdim = c.F if layer < 2 else c.NCLS
                    wcols = fdim + 8 if layer < 2 else fdim + 2
                    if True:
                        pA = psA.tile([P, c.F + 8], F32, tag="pA", space="PSUM")
                        if layer == 0:
                            xk = sbA.tile([P, c.KIN * P], F32, tag="xk")
                            nc.sync.dma_start(out=xk[:], in_=xT_d[t, :, :])
                            for k in range(c.KIN):
                                nc.tensor.matmul(
                                    out=pA[:, 0:wcols],
                                    lhsT=xk[:, k * P:(k + 1) * P],
                                    rhs=wcat0_t[:, k * wcols:(k + 1) * wcols],
                                    start=(k == 0), stop=(k == c.KIN - 1))
                        else:
                            hin = h_tiles[(layer, t)]
                            pT = psA.tile([P, P], F32, tag="pT", space="PSUM")
                            nc.tensor.transpose(out=pT[:], in_=hin[:],
                                                identity=ident_t[:])
                            hT = sbA.tile([P, P], F32, tag="hT")
                            nc.vector.tensor_copy(out=hT[:], in_=pT[:])
                            nc.tensor.matmul(
                                out=pA[:, 0:wcols], lhsT=hT[:],
                                rhs=(wcat1_t[:] if layer == 1 else wcat2_t[:]),
                                start=True, stop=True)
                        # keep er for this block resident in SBUF (fp16): the
                        # edge phase broadcasts it to edge slots via matmul
                        nh2 = c.H if layer < 2 else 1
                        ersb = hpool.tile([P, 8], FP16, tag=f"er{layer}_{t}")
                        nc.vector.tensor_copy(
                            out=ersb[:, 0:nh2],
                            in_=pA[:, fdim + nh2:fdim + 2 * nh2])
                        h_tiles[(f"er{layer}", t)] = ersb
                        fdim_ = fdim
                        nsl = fdim_ + 2 * (wcols - fdim_)  # fp16 slots used
                        fea = sbA.tile([P, c.F + 16], FP16, tag="fea")
                        nc.vector.tensor_copy(out=fea[:, 0:fdim_], in_=pA[:, 0:fdim_])
                        nc.vector.tensor_copy(
                            out=fea[:, fdim_:nsl].bitcast(F32),
                            in_=pA[:, fdim_:wcols])
                        nc.sync.dma_start(out=fe_loc[layer][t * P:(t + 1) * P, 0:nsl],
                                          in_=fea[:, 0:nsl])

                def phase_c(layer, after_block=None):
                    fdim = c.F if layer < 2 else c.NCLS      # 128 / 64
                    nh = c.H if layer < 2 else 1
                    rb = ROW if layer < 2 else ROW2
                    ecol = fdim  # el col offset within row
                    KP16 = K * P // 16
                    for b in range(c.NBLK):
                        idx = sbC.tile([P, K * 8], I16, tag="idx")
                        nc.sync.dma_start(out=idx[:], in_=eidx_d[b, :, :])
                        dstv = sbC.tile([P, K], F32, tag="dstv")
                        nc.sync.dma_start(out=dstv[:], in_=edstv_d[b, :, :])
                        stb = sbC.tile([P, KP16], I16, tag="stb")
                        nc.sync.dma_start(out=stb[:], in_=stbit_d[b, :, :])
                        feg = sbC.tile([P, K * ROW], FP16, tag="feg")
                        feg3 = feg[:, 0:K * rb].rearrange("p (k r) -> p k r", r=rb)
                        for gs, gnk, gbase in c.GROUPS:
                            nc.gpsimd.dma_gather(
                                feg3[:, gs:gs + gnk, :],
                                fe_full[layer][gbase:c.NG, :],
                                idx[:, gs * 8:(gs + gnk) * 8],
                                gnk * P, gnk * P, rb, queue_num=nextq())
                        # unpack transposed one-hot St[v, (k,p)] from bits
                        andt = sbC.tile([P, K * P], I16, tag="andt")
                        nc.vector.tensor_tensor(
                            out=andt[:].rearrange("p (j e) -> p j e", e=16),
                            in0=stb[:].unsqueeze(2).to_broadcast([P, KP16, 16]),
                            in1=bitc_t[:].unsqueeze(1).to_broadcast([P, KP16, 16]),
                            op=OP.bitwise_and)
                        sth = sbC.tile([P, K * P], FP16, tag="sth")
                        nc.vector.tensor_tensor(
                            out=sth[:].rearrange("p (j e) -> p j e", e=16),
                            in0=andt[:].rearrange("p (j e) -> p j e", e=16),
                            in1=bitc_t[:].unsqueeze(1).to_broadcast([P, KP16, 16]),
                            op=OP.is_equal)
                        # er_edge[p, (k,h)] = er_blk[dstv[p,k], h] via K matmuls
                        ersb = h_tiles[(f"er{layer}", b)]
                        erp = psC.tile([P, K * c.H], F32, tag="erp", space="PSUM")
                        for k in range(K):
                            nc.tensor.matmul(
                                out=erp[:, k * nh:k * nh + nh],
                                lhsT=sth[:, k * P:(k + 1) * P],
                                rhs=ersb[:, 0:nh], start=True, stop=True)
                        # f32 view of el (in feg)
                        el_f32 = feg3[:, :, fdim:fdim + 2 * nh].bitcast(F32)
                        # S[p, k, v] = (v == dstv[p, k])
                        s_all = sbC.tile([P, K * P], FP16, tag="sall")
                        nc.vector.tensor_tensor(
                            out=s_all[:].rearrange("p (k v) -> p k v", v=P),
                            in0=iota_t[:].unsqueeze(1).to_broadcast([P, K, P]),
                            in1=dstv[:].unsqueeze(2).to_broadcast([P, K, P]),
                            op=OP.is_equal)
                        # e = lrelu(el + er); ex = exp(e) overwrites el cols in feg
                        e_t = sbC.tile([P, K * c.H], F32, tag="e_t")
                        e3 = e_t[:, 0:K * nh].rearrange("p (k h) -> p k h", h=nh)
                        nc.vector.tensor_tensor(
                            out=e3, in0=el_f32,
                            in1=erp[:, 0:K * nh].rearrange("p (k h) -> p k h", h=nh),
                            op=OP.add)
                        nc.vector.scalar_tensor_tensor(
                            out=e_t[:, 0:K * nh], in0=e_t[:, 0:K * nh], scalar=GAT_NEG,
                            in1=e_t[:, 0:K * nh], op0=OP.mult, op1=OP.max)
                        nc.scalar.activation(out=feg3[:, :, ecol:ecol + nh], in_=e3,
                                             func=AF.Exp)
                        # exe = exp(e) pre-expanded along D on the (idle) scalar
                        # engine so the g-multiply is a contiguous fp16 DVE op
                        dph = c.D if layer < 2 else fdim
                        exe = sbC.tile([P, K * c.F], FP16, tag="exe")
                        exe4 = exe[:, 0:K * fdim].rearrange(
                            "p (k h d) -> p k h d", k=K, d=dph)
                        nc.scalar.activation(
                            out=exe4,
                            in_=e3.unsqueeze(3).to_broadcast([P, K, nh, dph]),
                            func=AF.Exp)
                        # g = f * ex, in place
                        nc.vector.tensor_tensor(
                            out=feg3[:, :, 0:fdim], in0=feg3[:, :, 0:fdim],
                            in1=exe[:, 0:K * fdim].rearrange(
                                "p (k d) -> p k d", k=K),
                            op=OP.mult)
                        gw = fdim + nh
                        agg = psC.tile([P, c.F + c.H], F32, tag="agg", space="PSUM")
                        for k in range(K):
                            nc.tensor.matmul(
                                out=agg[:, 0:gw],
                                lhsT=s_all[:, k * P:(k + 1) * P],
                                rhs=feg[:, k * rb:k * rb + gw],
                                start=(k == 0), stop=(k == K - 1))
                        rec = sbC.tile([P, c.H], F32, tag="rec")
                        nc.vector.tensor_scalar_max(rec[:, 0:nh],
                                                    agg[:, fdim:fdim + nh], 1e-30)
                        nc.vector.reciprocal(out=rec[:, 0:nh], in_=rec[:, 0:nh])
                        if layer < 2:
                            z = sbC.tile([P, c.F], F32, tag="z")
                            for h in range(nh):
                                nc.vector.tensor_scalar_mul(
                                    z[:, h * c.D:(h + 1) * c.D],
                                    agg[:, h * c.D:(h + 1) * c.D], rec[:, h:h + 1])
                            zm = sbC.tile([P, c.F], F32, tag="zm")
                            nc.vector.tensor_scalar_min(zm[:], z[:], 0.0)
                            ze = sbC.tile([P, c.F], F32, tag="ze")
                            nc.scalar.activation(out=ze[:], in_=zm[:], func=AF.Exp)
                            nc.vector.tensor_scalar_max(z[:], z[:], 0.0)
                            hout = hpool.tile([P, c.F], F32, tag=f"h{layer + 1}_{b}")
                            nc.vector.scalar_tensor_tensor(
                                out=hout[:], in0=ze[:], scalar=-1.0, in1=z[:],
                                op0=OP.add, op1=OP.add)
                            h_tiles[(layer + 1, b)] = hout
                        else:
                            lgt = sbC.tile([P, c.NCLS], F32, tag="lgt")
                            nc.vector.tensor_scalar_mul(lgt[:], agg[:, 0:c.NCLS],
                                                        rec[:, 0:1])
                            lgb = sbC.tile([P, c.NCLS], FP16, tag="lgb")
                            nc.vector.tensor_copy(out=lgb[:], in_=lgt[:])
                            nc.sync.dma_start(
                                out=lg_loc[b * 64:(b + 1) * 64, :]
                                    .rearrange("r (a d) -> (r a) d", a=2),
                                in_=lgb[:])
                        if after_block is not None:
                            after_block(b)

                def next_layer_cb(nl):
                    def cb(b):
                        phase_a_block(nl, b)
                        emit_ag(nl, b)
                    return cb

                def lg_cb(b):
                    # chunked AllGather of the logit table (chunk-major)
                    for j0, nb, cum in _chunk_spans(c.NBLK, AGCH_LG):
                        if b == j0 + nb - 1:
                            src = lg_loc[j0 * 64:(j0 + nb) * 64, :]
                            dst = lg_full[cum * c.C * 64:
                                          (cum + nb) * c.C * 64, :]
                            nc.gpsimd.collective_compute(
                                "AllGather", OP.bypass,
                                replica_groups=groups,
                                ins=[src.opt()], outs=[dst.opt()])

                if max_stage >= 1:
                    for t in range(c.NBLK):
                        phase_a_block(0, t)
                        emit_ag(0, t)
                if max_stage >= 2:
                    phase_c(0, after_block=next_layer_cb(1))
                if max_stage >= 4:
                    phase_c(1, after_block=next_layer_cb(2))
                if max_stage >= 6:
                    phase_c(2, after_block=lg_cb)

            # ---------------- CNN head ----------------
            with ExitStack() as dctx:
                sbD = dctx.enter_context(tc.tile_pool(name="sbD", bufs=2))
                sbD3 = dctx.enter_context(tc.tile_pool(name="sbD3", bufs=4))
                psD = dctx.enter_context(tc.tile_pool(name="psD", bufs=2, space="PSUM"))

                for st in range(c.ST if max_stage >= 8 else 0):
                    cidx = sbD.tile([P, KC * 8], I16, tag="cidx")
                    nc.sync.dma_start(out=cidx[:], in_=cidx_d[st, :, :])
                    cmsk = sbD.tile([P, 2 * KC], FP16, tag="cmsk")
                    nc.sync.dma_start(out=cmsk[:], in_=cmsk_d[st, :, :])
                    for g in range(KC // 8 if max_stage >= 9 else 0):
                        o = g * 8
                        # per-group gather + select so group g's compute
                        # overlaps group g+1's gather DMA
                        Xg = sbD3.tile([P, 8 * P], FP16, tag=f"X{g}")
                        X3 = Xg[:].rearrange("p (k r) -> p k r", r=P)
                        nc.gpsimd.dma_gather(
                            X3[:, 0:8, :], lg_full[:, :],
                            cidx[:, o * 8:(o + 8) * 8], 8 * P, 8 * P, P,
                            queue_num=nextq())
                        X64 = sbD3.tile([P, 8 * 64], FP16, tag=f"X64{g}")
                        X64_3 = X64[:].rearrange("p (k r) -> p k r", r=64)
                        Xo = sbD3.tile([P, 8 * 64], FP16, tag=f"Xo{g}")
                        Xo_3 = Xo[:].rearrange("p (k r) -> p k r", r=64)
                        # exact parity select: X64 = Xl*(1-m) + Xr*m (masks 0/1)
                        nc.vector.tensor_tensor(
                            out=X64_3, in0=X3[:, :, 0:64],
                            in1=cmsk[:, o:o + 8].unsqueeze(2)
                                .to_broadcast([P, 8, 64]),
                            op=OP.mult)
                        nc.vector.tensor_tensor(
                            out=Xo_3, in0=X3[:, :, 64:P],
                            in1=cmsk[:, KC + o:KC + o + 8].unsqueeze(2)
                                .to_broadcast([P, 8, 64]),
                            op=OP.mult)
                        nc.vector.tensor_tensor(out=X64[:], in0=X64[:],
                                                in1=Xo[:], op=OP.add)
                        y2p = psD.tile([64, 512], F32, tag="y2p", space="PSUM")
                        for par in range(2):
                            y1p = psD.tile([CH3, 512], F32, tag=f"y1p{par}",
                                           space="PSUM")
                            nc.tensor.matmul(
                                out=y1p[:],
                                lhsT=cw1ab_t[par * 64:(par + 1) * 64,
                                             par * CH3:(par + 1) * CH3],
                                rhs=X64[par * 64:(par + 1) * 64, :],
                                start=True, stop=True)
                            y1s = sbD3.tile([CH3, 512], FP16, tag=f"y1s{par}")
                            if use_act_lrelu:
                                nc.scalar.activation(out=y1s[:], in_=y1p[:],
                                                     func=AF.Lrelu,
                                                     bias=cb1r_t[:, 0:1],
                                                     alpha=CNN_NEG)
                            else:
                                yt = sbD3.tile([CH3, 512], F32, tag=f"yt{par}")
                                nc.vector.tensor_scalar_add(yt[:], y1p[:],
                                                            cb1r_t[:, 0:1])
                                nc.vector.scalar_tensor_tensor(
                                    out=y1s[:], in0=yt[:], scalar=CNN_NEG, in1=yt[:],
                                    op0=OP.mult, op1=OP.max)
                            nc.tensor.matmul(out=y2p[par * 32:(par + 1) * 32, :],
                                             lhsT=w2c_t[:], rhs=y1s[:],
                                             start=True, stop=True,
                                             tile_position=(0, par * 32))
                        ysb = sbD.tile([64, 512], F32, tag="ysb")
                        if use_act_lrelu:
                            nc.scalar.activation(out=ysb[:], in_=y2p[:], func=AF.Lrelu,
                                                 bias=cb2_t[0:64, 0:1], alpha=CNN_NEG)
                        else:
                            nc.vector.tensor_scalar_add(ysb[:], y2p[:],
                                                        cb2_t[0:64, 0:1])
                            nc.vector.scalar_tensor_tensor(
                                out=ysb[:], in0=ysb[:], scalar=CNN_NEG, in1=ysb[:],
                                op0=OP.mult, op1=OP.max)
                        r0 = st * 64 + g * 16
                        nc.sync.dma_start(
                            out=y_d[r0:r0 + 16, :].rearrange("(b a) d -> a b d", a=2),
                            in_=ysb[0:64:32, :].rearrange("a (b d) -> a b d", d=c.NCLS))
    # Align SWDGE queue assignment with Tile's DMASW sem-lane rotation:
    # sem lane s must always service the same queue, so queue = lane % 4.
    from concourse.tile_sem_assignment import PROC_NAME_TO_IDX
    lane_to_q = {}
    for q in range(8):
        if f"DMASW{q}" in PROC_NAME_TO_IDX:
            lane_to_q[PROC_NAME_TO_IDX[f"DMASW{q}"]] = q % 4
    nfix = 0
    for blk in nc.m.functions[0].blocks:
        for inst in blk.instructions:
            if isinstance(inst, mybir.InstDMAGatherAnt):
                proc = inst.bass_scheduled_proc
                if proc in lane_to_q:
                    inst.queue_num = lane_to_q[proc]
                    nfix += 1
    nc.compile()
    return nc


# ------------------------------------------------------------------
# host-side preparation
# ------------------------------------------------------------------

def _block_diag_a(a):
    a = np.asarray(a, np.float32)
    hh, dd = a.shape
    out = np.zeros((hh * dd, hh), np.float32)
    for h in range(hh):
        out[h * dd:(h + 1) * dd, h] = a[h]
    return out


def _wrap16(slots, ncols):
    """slots: int array [NI] -> [128, NI//16] int16, wrapped in 16 partitions
    and replicated into all 8 16-partition groups."""
    ni = slots.shape[0]
    m = np.zeros((P, ncols), np.int16)
    j = np.arange(ni)
    for rep in range(8):
        m[j % 16 + 16 * rep, j // 16] = slots.astype(np.int16)
    return m


def compute_groups(nloc, dst, ncores):
    """f-gather call plan from the worst per-block edge count.

    Calls are capped at 1024 indices (8 chunks) by the gather ucode ring.
    Call 0 addresses table rows [0, 32768), call 1 rows [NG-32768, NG);
    overflow calls address [0, 32768) again (they only ever hold edges
    whose src row < 32768).
    """
    nblk = (nloc + P - 1) // P
    worst = 0
    for core in range(ncores):
        lo = core * nloc
        m = (dst >= lo) & (dst < lo + nloc)
        cnt = np.bincount((dst[m] - lo) // P, minlength=nblk)
        worst = max(worst, int(cnt.max()))
    npad = nblk * P
    base2 = ncores * npad - 32768
    if worst <= 1024:
        return [(0, (worst + P - 1) // P, 0)], base2
    groups = [(0, 8, 0), (8, 8, base2)]
    left = worst - 2048
    gs = 16
    while left > 0:
        gnk = min(8, (left + P - 1) // P)
        groups.append((gs, gnk, 0))
        gs += gnk
        left -= gnk * P
    return groups, base2


def prepare_inputs(cfg, x, src, dst, paths, W0, al0, ar0, W1, al1, ar1,
                   W2, al2, ar2, cw1, cb1, cw2, cb2):
    c = cfg
    x = np.asarray(x, np.float32)
    src = np.asarray(src, np.int64)
    dst = np.asarray(dst, np.int64)
    paths = np.asarray(paths, np.int64)
    W0 = np.asarray(W0, np.float32)
    W1 = np.asarray(W1, np.float32)
    W2 = np.asarray(W2, np.float32)
    al2 = np.asarray(al2, np.float32)
    ar2 = np.asarray(ar2, np.float32)
    cw1 = np.asarray(cw1, np.float32)
    cw2 = np.asarray(cw2, np.float32)

    wcat0 = np.concatenate(
        [W0, W0 @ _block_diag_a(al0), W0 @ _block_diag_a(ar0)], axis=1
    ).astype(np.float32).reshape(c.KIN, P, c.F + 8)
    wcat1 = np.ascontiguousarray(np.concatenate(
        [W1, W1 @ _block_diag_a(al1), W1 @ _block_diag_a(ar1)], axis=1
    ).astype(np.float32))
    wcat2 = np.ascontiguousarray(np.concatenate(
        [W2, W2 @ al2.reshape(-1, 1), W2 @ ar2.reshape(-1, 1)], axis=1
    ).astype(np.float32))

    ch = c.CNN_CH
    CH3 = 3 * ch
    cw1e = np.zeros((50, CH3), np.float32)
    for co in range(ch):
        for pp in range(3):
            for ci in range(10):
                for t in range(3):
                    cw1e[ci * 5 + pp + t, co * 3 + pp] = cw1[co, ci, t, 0]
    cw1ab = np.zeros((P, 2 * CH3), np.float32)
    cw1ab[0:50, 0:CH3] = cw1e
    cw1ab[64:114, CH3:2 * CH3] = cw1e
    cb1r = np.repeat(np.asarray(cb1, np.float32), 3).reshape(CH3, 1)
    w2c = np.zeros((CH3, 32), np.float32)
    for co in range(ch):
        for pp in range(3):
            w2c[co * 3 + pp, :] = cw2[0, co, pp, 0]
    cb2a = np.full((P, 1), np.float32(np.asarray(cb2).reshape(-1)[0]))

    grow = _chunk_map(c.NBLK, AGCH_FE, c.NLOC, c.C, P)
    lgmap = _chunk_map(c.NBLK, AGCH_LG, c.NLOC, c.C, 64)

    in_maps = []
    for core in range(c.C):
        lo, hi = core * c.NLOC, (core + 1) * c.NLOC
        xs = np.zeros((c.NPAD, c.IN_DIM), np.float32)
        xs[:c.NLOC] = x[lo:hi]
        # per-block [q, (k, n)] layout so each block loads as one
        # contiguous 128x2KB DMA
        xT = np.ascontiguousarray(
            xs.reshape(c.NBLK, P, c.KIN, P).transpose(0, 3, 2, 1)
        ).reshape(c.NBLK, P, c.KIN * P)

        m = (dst >= lo) & (dst < hi)
        es, ed = grow(src[m]), dst[m] - lo
        order = np.argsort(ed, kind="stable")
        es, ed = es[order], ed[order]
        eblk = ed // P
        eidx = np.zeros((c.NBLK, P, c.K * 8), np.int16)
        edstv = np.full((c.NBLK, P, c.K), -1.0, np.float32)
        stbit = np.zeros((c.NBLK, P, c.K * P // 16), np.uint16)
        base2 = c.NG - 32768  # second-window base; window = [base2, NG)
        for b in range(c.NBLK):
            bm = eblk == b
            bs, bd = es[bm], ed[bm]
            # assign edges to gather calls: call 0 covers rows [0, 32768),
            # call 1 covers [base2, NG), extra calls cover [0, 32768).
            caps = [gnk * P for _, gnk, _ in c.GROUPS]
            if len(c.GROUPS) == 1:
                assert len(bs) <= caps[0]
                asn = [np.arange(len(bs))]
            else:
                i1 = np.where(bs >= 32768)[0]       # must use window-1 call
                i0 = np.where(bs < base2)[0]        # must use a base-0 call
                iflex = np.where((bs >= base2) & (bs < 32768))[0]
                assert len(i1) <= caps[1], (core, b, len(i1))
                nf2 = min(caps[1] - len(i1), len(iflex))
                asn = [None] * len(c.GROUPS)
                asn[1] = np.concatenate([i1, iflex[:nf2]])
                rest = np.concatenate([i0, iflex[nf2:]]).astype(np.int64)
                asn[0] = rest[:caps[0]]
                rest = rest[caps[0]:]
                for gi in range(2, len(c.GROUPS)):
                    asn[gi] = rest[:caps[gi]]
                    rest = rest[caps[gi]:]
                assert rest.size == 0, (core, b, rest.size)
            for gi, (gstart, gnk, gbase) in enumerate(c.GROUPS):
                sel = asn[gi]
                hs, hd = bs[sel], bd[sel]
                ne = hs.shape[0]
                # pad slots point at spread-out (valid) rows: same-address
                # row-0 fetches serialize on one HBM bank and skew the DMA
                # engine load badly
                g_idx = (np.arange(gnk * P, dtype=np.int64) * 997) % 16384
                g_idx[:ne] = hs - gbase
                assert ne == 0 or (g_idx[:ne].min() >= 0
                                   and g_idx[:ne].max() < 32768)
                j = np.arange(ne)
                # slot j of this call -> chunk gstart + j//128, partition j%128
                hdr = (hd - b * P).astype(np.int64)
                edstv[b, j % P, gstart + j // P] = hdr.astype(np.float32)
                slot = (gstart + j // P) * P + j % P
                np.bitwise_or.at(
                    stbit[b], (hdr, slot >> 4),
                    (np.uint16(1) << (slot & 15).astype(np.uint16)))
                eidx[b, :, gstart * 8:(gstart + gnk) * 8] = _wrap16(
                    g_idx, gnk * 8)


        pl = np.zeros((c.NPAD, 50), np.int64)
        pl[:c.NLOC] = paths[lo:hi].reshape(c.NLOC, 50)
        pgp = lgmap(pl)                                      # pair row
        ppar = (pl % c.NLOC) & 1                             # node parity
        cidx = np.zeros((c.ST, P, c.KC * 8), np.int16)
        cmsk = np.zeros((c.ST, P, 2 * c.KC), np.float16)
        for st in range(c.ST):
            j = np.arange(c.KC * P)
            k, p = j // P, j % P
            n = st * 64 + 2 * k + (p >= 64)
            q = p % 64
            valid = q < 50
            qm = np.minimum(q, 49)
            slots = np.where(valid, pgp[n, qm], (j * 1997) % (c.NG // 2))
            par = np.where(valid, ppar[n, qm], 0)
            cidx[st] = _wrap16(slots, c.KC * 8)
            cmsk[st, p, k] = (1 - par).astype(np.float16)
            cmsk[st, p, c.KC + k] = par.astype(np.float16)
        in_maps.append({
            "xT": xT, "wcat0": wcat0, "wcat1": wcat1, "wcat2": wcat2,
            "eidx": eidx, "edstv": edstv, "stbit": stbit.view(np.int16),
            "bitc": np.broadcast_to(
                (np.uint16(1) << np.arange(16, dtype=np.uint16))
                .view(np.int16), (P, 16)).copy(),
            "cidx": cidx, "cmsk": cmsk,
            "cw1ab": cw1ab.astype(np.float16), "cb1r": cb1r,
            "w2c": w2c.astype(np.float16), "cb2": cb2a,
        })
    return in_maps


_CACHE = {}


def kernel(**inputs):
    x = np.asarray(inputs["x"], np.float32)
    n_nodes = x.shape[0]
    dst = np.asarray(inputs["dst"], np.int64)
    ncores = 8
    nloc = n_nodes // ncores
    groups, _ = compute_groups(nloc, dst, ncores)
    cfg = Cfg(n_nodes=n_nodes, n_cores=ncores, in_dim=x.shape[1],
              n_hid=32, h_hid=4, n_cls=64,
              cnn_ch=int(np.asarray(inputs["cw1"]).shape[0]),
              groups=groups, kc=32)

    key = (n_nodes, cfg.IN_DIM, tuple(groups))
    if key not in _CACHE:
        _CACHE[key] = build_program(cfg)
    nc = _CACHE[key]

    in_maps = prepare_inputs(cfg, **inputs)
    res = bass_utils.run_bass_kernel_spmd(nc, in_maps, core_ids=list(range(cfg.C)))
    y = np.concatenate(
        [res.results[core]["y"][:cfg.NLOC] for core in range(cfg.C)], axis=0)
    return np.ascontiguousarray(y.astype(np.float32))

